# revision 1
# baseline (speedup 1.0000x reference)
"""Causal self-attention (b=2, t=2048, d_model=1024, 16 heads) on 8 trn2 cores.

Sharding: tensor-parallel over heads (2 heads per core). Each core computes
qkv = x @ W_qkv[:, head-slice], attention for its heads, and a partial
out_heads @ W_proj[head-rows, :]. The 8 partial [4096, 1024] outputs are
summed on the host (the all-reduce after proj), plus b_proj.

Device layout notes:
- Host pre-transposes x to xT [1024, 4096] so the d_model contraction dim is
  on partitions for every matmul; no on-device input transposes are needed.
- Stage A computes Q^T/K^T/V^T = W.T @ xT with both heads stacked on the
  partition axis ([128] = 2 heads x 64 dims).
- V^T is PE-transposed back to V [rows, 2x(64+1)] with a ones column per head
  so the att@V matmul also accumulates the softmax denominator row.
- Scores are computed transposed (sT[k, q]); softmax needs no max-subtraction
  (logits ~ N(0,1), exp cannot overflow fp32).
- Causality: k-tiles above the diagonal are skipped, diagonal tiles compute
  only the valid column suffix, and one 128x128 upper-triangular mask
  multiply fixes the diagonal band.
- All matmuls run in float32r (full PE rate at N>=256, ~1e-4 relative error).
"""

import sys

sys.path.insert(0, "/opt/trn_rl_repo")

import numpy as np

import concourse.bass as bass  # noqa: F401
import concourse.tile as tile
from concourse import bacc, mybir

F32 = mybir.dt.float32
F32R = mybir.dt.float32r
BF16 = mybir.dt.bfloat16
DT_AT = BF16   # operand dtype for stage A matmuls and attention (B/C)
EXP = mybir.ActivationFunctionType.Exp
IDENT = mybir.ActivationFunctionType.Identity

B = 2
T = 2048
DM = 1024
NH = 16
HD = 64
ROWS = B * T            # 4096
NCORES = 8
HPC = NH // NCORES      # heads per core = 2
WCOLS = HPC * HD        # 128 qkv columns per core for each of q/k/v
QCH = 512               # query chunk
KTILE = 128             # key tile
NQC = T // QCH          # 4 query chunks per batch
NKT_B = T // KTILE      # 16 key tiles per batch
NRC = ROWS // QCH       # 8 row chunks
NKD = DM // 128         # 8 d_model k-tiles
VW = 2 * (HD + 1)       # 130: V block width (2 heads x (64 dims + ones col))


class _Alloc:
    """Tag-based routing to the right tile pool."""
    WORK = {"xt", "ea", "eb", "bc", "osb", "rc2"}
    WORK_BUFS = {"xt": 10, "ea": 6, "eb": 4, "bc": 2, "osb": 3, "rc2": 4}

    def __init__(self, pers, work, ps, pso):
        self.pers, self.work, self.ps, self.pso = pers, work, ps, pso

    def tile(self, shape, dt, tag):
        if tag in ("ps", "ps2"):
            return self.ps.tile(shape, dt, tag=tag, name=tag)
        if tag == "pso":
            return self.pso.tile(shape, dt, tag=tag, name=tag)
        if tag in self.WORK:
            return self.work.tile(shape, dt, tag=tag, name=tag, bufs=self.WORK_BUFS[tag])
        return self.pers.tile(shape, dt, tag=tag, name=tag)


def _emit_consts(nc, al, aps):
    (xt_d, wq_d, wk_d, wv_d, wp_d, bq_d, bk_d, bv_d, triu_d, e2_d, id_d,
     vones_d, out_d) = aps
    C = {}
    C["qt"] = al.tile([128, ROWS], DT_AT, tag="qt")
    C["kt"] = al.tile([128, ROWS], DT_AT, tag="kt")
    C["vt"] = al.tile([128, ROWS], DT_AT, tag="vt")
    C["v"] = al.tile([128, (ROWS // 128) * VW], DT_AT, tag="v")
    C["ot"] = al.tile([128, ROWS], F32R, tag="ot")
    C["wq"] = al.tile([128, DM], DT_AT, tag="wq")
    C["wk"] = al.tile([128, DM], DT_AT, tag="wk")
    C["wv"] = al.tile([128, DM], DT_AT, tag="wv")
    C["wp"] = al.tile([128, DM], F32R, tag="wp")
    C["bq"] = al.tile([128, 1], F32, tag="bq")
    C["bk"] = al.tile([128, 1], F32, tag="bk")
    C["bv"] = al.tile([128, 1], F32, tag="bv")
    C["triu"] = al.tile([128, 128], DT_AT, tag="triu")
    C["triu2"] = al.tile([128, 256], DT_AT, tag="triu2")
    C["e2"] = al.tile([2, 128], F32R, tag="e2")
    C["id"] = al.tile([128, 128], DT_AT, tag="id")
    for k in range(NKD):
        nc.sync.dma_start(C["wq"][:, k * 128:(k + 1) * 128], wq_d[k * 128:(k + 1) * 128, :])
        nc.sync.dma_start(C["wk"][:, k * 128:(k + 1) * 128], wk_d[k * 128:(k + 1) * 128, :])
        nc.sync.dma_start(C["wv"][:, k * 128:(k + 1) * 128], wv_d[k * 128:(k + 1) * 128, :])
    nc.sync.dma_start(C["wp"][:], wp_d[:])
    nc.sync.dma_start(C["bq"][:], bq_d[:])
    nc.sync.dma_start(C["bk"][:], bk_d[:])
    nc.sync.dma_start(C["bv"][:], bv_d[:])
    nc.sync.dma_start(C["triu"][:], triu_d[:])
    nc.sync.dma_start(C["triu2"][:, 0:128], triu_d[:])
    nc.sync.dma_start(C["triu2"][:, 128:256], triu_d[:])
    nc.sync.dma_start(C["e2"][:], e2_d[:])
    nc.sync.dma_start(C["id"][:], id_d[:])
    # ones columns of the V blocks (cols 64 and 129 of each 130-block)
    v_blocks = C["v"].rearrange("p (i w) -> p i w", w=VW)
    nc.sync.dma_start(v_blocks[:, :, HD], vones_d[:])
    nc.sync.dma_start(v_blocks[:, :, 2 * HD + 1], vones_d[:])
    return C


def _emit_body(nc, al, aps, C, parts=("a", "bc", "d")):
    (xt_d, wq_d, wk_d, wv_d, wp_d, bq_d, bk_d, bv_d, triu_d, e2_d, id_d,
     vones_d, out_d) = aps
    qt_sb, kt_sb, vt_sb, v_sb, ot_sb = C["qt"], C["kt"], C["vt"], C["v"], C["ot"]
    wq_sb, wk_sb, wv_sb, wp_sb = C["wq"], C["wk"], C["wv"], C["wp"]
    bq_sb, bk_sb, bv_sb = C["bq"], C["bk"], C["bv"]
    triu_sb, e2_sb, id_sb = C["triu"], C["e2"], C["id"]

    if "a" in parts:
        _emit_stage_a(nc, al, aps, C)
    if "bc" in parts:
        _emit_attn(nc, al, aps, C, do_d=("d" in parts))
    elif "d" in parts:
        _emit_proj_all(nc, al, aps, C)


def _emit_stage_a(nc, al, aps, C):
    (xt_d, wq_d, wk_d, wv_d, wp_d, bq_d, bk_d, bv_d, triu_d, e2_d, id_d,
     vones_d, out_d) = aps
    qt_sb, kt_sb, vt_sb, v_sb, ot_sb = C["qt"], C["kt"], C["vt"], C["v"], C["ot"]
    wq_sb, wk_sb, wv_sb, wp_sb = C["wq"], C["wk"], C["wv"], C["wp"]
    bq_sb, bk_sb, bv_sb = C["bq"], C["bk"], C["bv"]
    triu_sb, e2_sb, id_sb = C["triu"], C["e2"], C["id"]

    # ---- stage A: qkvT = W.T @ xT (+bias), then V^T -> V transposes ----
    for rc in range(NRC):
        cs = rc * QCH
        slot1 = al.tile([128, 2 * QCH], F32, tag="ps2")
        slot2 = al.tile([128, 2 * QCH], F32, tag="ps2")
        psq = slot1[:, 0:QCH]
        psk = slot1[:, QCH:2 * QCH]
        psv = slot2[:, 0:QCH]
        for k in range(NKD):
            xt_t = al.tile([128, QCH], DT_AT, tag="xt")
            nc.gpsimd.dma_start(xt_t[:], xt_d[k * 128:(k + 1) * 128, cs:cs + QCH])
            st = (k == 0)
            sp = (k == NKD - 1)
            nc.tensor.matmul(psq, wq_sb[:, k * 128:(k + 1) * 128], xt_t[:], start=st, stop=sp)
            nc.tensor.matmul(psk, wk_sb[:, k * 128:(k + 1) * 128], xt_t[:], start=st, stop=sp)
            nc.tensor.matmul(psv, wv_sb[:, k * 128:(k + 1) * 128], xt_t[:], start=st, stop=sp)
        nc.scalar.activation(qt_sb[:, cs:cs + QCH], psq, IDENT, bias=bq_sb[:])
        nc.scalar.activation(kt_sb[:, cs:cs + QCH], psk, IDENT, bias=bk_sb[:])
        nc.scalar.activation(vt_sb[:, cs:cs + QCH], psv, IDENT, bias=bv_sb[:])

    # V^T -> V transposes as a separate pass; each transpose gets its own
    # psum slot so a PE transpose write never shares a bank with a DVE read
    # of the previous transpose (same-bank PE-W + DVE-R is a HW hazard).
    for i in range(ROWS // 128):
        tslot = al.tile([128, 2 * QCH], F32, tag="ps2")
        pst = tslot[:, 0:64].bitcast(DT_AT)
        nc.tensor.transpose(pst, vt_sb[:, i * 128:(i + 1) * 128], id_sb[:])
        nc.vector.tensor_copy(v_sb[:, i * VW:i * VW + HD], pst[:, 0:HD])
        nc.vector.tensor_copy(v_sb[:, i * VW + HD + 1:i * VW + 2 * HD + 1], pst[:, HD:128])


def _emit_proj_all(nc, al, aps, C):
    (xt_d, wq_d, wk_d, wv_d, wp_d, bq_d, bk_d, bv_d, triu_d, e2_d, id_d,
     vones_d, out_d) = aps
    ot_sb, wp_sb = C["ot"], C["wp"]
    for qt in range(ROWS // 128):
        q0 = qt * 128
        osb = al.tile([128, DM], F32, tag="osb")
        for ct in range(DM // 512):
            psp = al.tile([128, 512], F32, tag="ps")
            nc.tensor.matmul(psp[:], ot_sb[:, q0:q0 + 128],
                             wp_sb[:, ct * 512:(ct + 1) * 512])
            nc.vector.tensor_copy(osb[:, ct * 512:(ct + 1) * 512], psp[:])
        nc.sync.dma_start(out_d[q0:q0 + 128, :], osb[:])


def _emit_attn(nc, al, aps, C, do_d=True):
    (xt_d, wq_d, wk_d, wv_d, wp_d, bq_d, bk_d, bv_d, triu_d, e2_d, id_d,
     vones_d, out_d) = aps
    qt_sb, kt_sb, vt_sb, v_sb, ot_sb = C["qt"], C["kt"], C["vt"], C["v"], C["ot"]
    wq_sb, wk_sb, wv_sb, wp_sb = C["wq"], C["wk"], C["wv"], C["wp"]
    bq_sb, bk_sb, bv_sb = C["bq"], C["bk"], C["bv"]
    triu_sb, e2_sb, id_sb = C["triu"], C["e2"], C["id"]
    triu2_sb = C["triu2"]

    # ---- stages B/C/D per (batch, query chunk) ----
    # Score matmuls get a priority boost so the PE instruction stream keeps
    # ~2 k-tiles of scores in flight ahead of the exp->AV chain (the
    # scheduler's cost model under-prices exp, so without this PE convoys).
    tc = al.tc
    for b in range(B):
        for qc in range(NQC):
            qglob = b * T + qc * QCH
            nkt = (qc + 1) * (QCH // KTILE)
            pso2 = al.tile([HD + 1, 2 * QCH], F32, tag="pso")
            pso_a = pso2[:, 0:QCH]
            pso_b = pso2[:, QCH:2 * QCH]
            for kt in range(nkt):
                r = kt * KTILE - qc * QCH
                s = max(0, r)          # valid column suffix start
                i = b * NKT_B + kt     # global 128-row tile index for K/V
                kcol = b * T + kt * KTILE
                ps2 = al.tile([128, 2 * QCH], F32, tag="ps2")
                nc.tensor.matmul(ps2[:, s:QCH], kt_sb[0:HD, kcol:kcol + KTILE],
                                 qt_sb[0:HD, qglob + s:qglob + QCH])
                nc.tensor.matmul(ps2[:, QCH + s:], kt_sb[HD:128, kcol:kcol + KTILE],
                                 qt_sb[HD:128, qglob + s:qglob + QCH])
                ea2 = al.tile([128, 2 * QCH], DT_AT, tag="ea")
                src_v = ps2.rearrange("p (h q) -> p h q", h=2)[:, :, s:]
                dst_v = ea2.rearrange("p (h q) -> p h q", h=2)[:, :, s:]
                nc.scalar.activation(dst_v, src_v, EXP, scale=0.125)
                if r >= 0:  # diagonal tile: triangular mask on the 128-col bands
                    band = ea2.rearrange("p (h q) -> p h q", h=2)[:, :, s:s + KTILE]
                    nc.vector.tensor_mul(band, band, triu2_sb[:].rearrange("p (h q) -> p h q", h=2))
                st = (kt == 0)
                sp = (kt == nkt - 1)
                nc.tensor.matmul(pso_a[:, s:], v_sb[:, i * VW:i * VW + HD + 1],
                                 ea2[:, s:QCH], start=st, stop=sp)
                nc.tensor.matmul(pso_b[:, s:], v_sb[:, i * VW + HD + 1:i * VW + VW],
                                 ea2[:, QCH + s:], start=st, stop=sp)
            # normalize by the accumulated denominator row (index HD)
            rca = al.tile([1, QCH], F32R, tag="rc2")
            rcb = al.tile([1, QCH], F32R, tag="rc2")
            with nc.allow_low_precision(reason="f32r softmax denom recip"):
                nc.vector.reciprocal(rca[:], pso_a[HD:HD + 1, :])
                nc.vector.reciprocal(rcb[:], pso_b[HD:HD + 1, :])
            psbc = al.tile([128, 2 * QCH], F32, tag="ps2")
            nc.tensor.matmul(psbc[0:HD, 0:QCH], e2_sb[0:1, 0:HD], rca[:])
            nc.tensor.matmul(psbc[0:HD, QCH:2 * QCH], e2_sb[0:1, 0:HD], rcb[:])
            bc2 = al.tile([HD, 2 * QCH], F32, tag="bc")
            nc.vector.tensor_copy(bc2[:], psbc[0:HD, :])
            nc.vector.tensor_mul(ot_sb[0:HD, qglob:qglob + QCH], pso_a[0:HD, :], bc2[:, 0:QCH])
            nc.vector.tensor_mul(ot_sb[HD:128, qglob:qglob + QCH], pso_b[0:HD, :], bc2[:, QCH:])
            # proj for this chunk's 4 query tiles
            for j in range(QCH // 128 if do_d else 0):
                q0 = qglob + j * 128
                osb = al.tile([128, DM], F32, tag="osb")
                psp = al.tile([128, 2 * QCH], F32, tag="ps2")
                for ct in range(DM // 512):
                    nc.tensor.matmul(psp[:, ct * 512:(ct + 1) * 512], ot_sb[:, q0:q0 + 128],
                                     wp_sb[:, ct * 512:(ct + 1) * 512])
                    nc.vector.tensor_copy(osb[:, ct * 512:(ct + 1) * 512],
                                          psp[:, ct * 512:(ct + 1) * 512])
                nc.sync.dma_start(out_d[q0:q0 + 128, :], osb[:])


def build_module(repeat=1, loop_n=0, parts=("a", "bc", "d"), pre_parts=()):
    nc = bacc.Bacc("TRN2", target_bir_lowering=False, debug=False,
                   enable_asserts=True, num_devices=NCORES)

    def din(name, shape, dt=F32R):
        return nc.dram_tensor(name, shape, dt, kind="ExternalInput").ap()

    aps = (
        din("xt", [DM, ROWS], DT_AT),
        din("wq", [DM, WCOLS], DT_AT),
        din("wk", [DM, WCOLS], DT_AT),
        din("wv", [DM, WCOLS], DT_AT),
        din("wp", [WCOLS, DM], F32R),
        din("bq", [WCOLS, 1], F32),
        din("bk", [WCOLS, 1], F32),
        din("bv", [WCOLS, 1], F32),
        din("triu", [128, 128], DT_AT),
        din("e2", [2, 128], F32R),
        din("ident", [128, 128], DT_AT),
        din("vones", [128, ROWS // 128], DT_AT),
        nc.dram_tensor("out", [ROWS, DM], F32, kind="ExternalOutput").ap(),
    )
    with tile.TileContext(nc) as tc:
        with tc.tile_pool(name="pers", bufs=1) as pers, \
             tc.tile_pool(name="work", bufs=4) as work, \
             tc.tile_pool(name="ps", bufs=3, space="PSUM") as psp, \
             tc.tile_pool(name="pso", bufs=1, space="PSUM") as psop:
            al = _Alloc(pers, work, psp, psop)
            al.tc = tc
            consts = _emit_consts(nc, al, aps)
            if pre_parts:
                _emit_body(nc, al, aps, consts, parts=pre_parts)
            if loop_n:
                with tc.For_i(0, loop_n, 1):
                    _emit_body(nc, al, aps, consts, parts=parts)
            else:
                for r in range(repeat):
                    _emit_body(nc, al, aps, consts, parts=parts)
    nc.compile()
    return nc


def _host_prep(x, W_qkv, b_qkv, W_proj):
    import ml_dtypes
    bf16 = ml_dtypes.bfloat16
    x = np.asarray(x, np.float32)
    W_qkv = np.asarray(W_qkv, np.float32)
    b_qkv = np.asarray(b_qkv, np.float32)
    W_proj = np.asarray(W_proj, np.float32)
    xt = np.ascontiguousarray(x.reshape(ROWS, DM).T.astype(bf16))
    triu = np.triu(np.ones((128, 128), bf16))
    e2 = np.zeros((2, 128), np.float32)
    e2[0, 0:HD] = 1.0
    e2[1, HD:128] = 1.0
    ident = np.eye(128, dtype=bf16)
    in_maps = []
    for c in range(NCORES):
        h0 = c * WCOLS  # first qkv column of this core's 2 heads
        in_maps.append({
            "xt": xt,
            "wq": np.ascontiguousarray(W_qkv[:, h0:h0 + WCOLS].astype(bf16)),
            "wk": np.ascontiguousarray(W_qkv[:, DM + h0:DM + h0 + WCOLS].astype(bf16)),
            "wv": np.ascontiguousarray(W_qkv[:, 2 * DM + h0:2 * DM + h0 + WCOLS].astype(bf16)),
            "wp": np.ascontiguousarray(W_proj[h0:h0 + WCOLS, :]),
            "bq": np.ascontiguousarray(b_qkv[h0:h0 + WCOLS, None]),
            "bk": np.ascontiguousarray(b_qkv[DM + h0:DM + h0 + WCOLS, None]),
            "bv": np.ascontiguousarray(b_qkv[2 * DM + h0:2 * DM + h0 + WCOLS, None]),
            "triu": triu,
            "e2": e2,
            "ident": ident,
            "vones": np.ones((128, ROWS // 128), bf16),
        })
    return in_maps


class _Runner:
    """Compile once, execute many times (mirrors bass2jax.run_bass_via_pjrt)."""

    def __init__(self, nc):
        import jax
        from jax.sharding import Mesh, PartitionSpec
        from jax.experimental.shard_map import shard_map
        from concourse import bass2jax
        from concourse import mybir as _mybir

        bass2jax.install_neuronx_cc_hook()
        self.jax = jax
        in_names, out_names, out_avals, zero_shapes = [], [], [], []
        partition_name = nc.partition_id_tensor.name if nc.partition_id_tensor else None
        for alloc in nc.m.functions[0].allocations:
            if not isinstance(alloc, _mybir.MemoryLocationSet):
                continue
            name = alloc.memorylocations[0].name
            if alloc.kind == "ExternalInput":
                if name != partition_name:
                    in_names.append(name)
            elif alloc.kind == "ExternalOutput":
                shape = tuple(alloc.tensor_shape)
                dtype = _mybir.dt.np(alloc.dtype)
                out_names.append(name)
                out_avals.append(jax.core.ShapedArray(shape, dtype))
                zero_shapes.append((shape, dtype))
        self.in_names = in_names
        self.out_names = out_names
        self.out_avals = out_avals
        self.zero_shapes = zero_shapes
        n_params = len(in_names)
        n_outs = len(out_avals)
        all_in_names = in_names + out_names + ([partition_name] if partition_name else [])

        def _body(*args):
            operands = list(args)
            if partition_name is not None:
                operands.append(bass2jax.partition_id_tensor())
            outs = bass2jax._bass_exec_p.bind(
                *operands,
                out_avals=tuple(out_avals),
                in_names=tuple(all_in_names),
                out_names=tuple(out_names),
                lowering_input_output_aliases=(),
                sim_require_finite=True,
                sim_require_nnan=True,
                nc=nc,
            )
            return tuple(outs)

        devices = jax.devices()[:NCORES]
        mesh = Mesh(np.asarray(devices), ("core",))
        self.mesh = mesh
        self.pspec = PartitionSpec("core")
        in_specs = (PartitionSpec("core"),) * (n_params + n_outs)
        out_specs = (PartitionSpec("core"),) * n_outs
        self.donate = tuple(range(n_params, n_params + n_outs))
        self.sharded = jax.jit(
            shard_map(_body, mesh=mesh, in_specs=in_specs, out_specs=out_specs,
                      check_rep=False),
            donate_argnums=self.donate, keep_unused=True)

    def concat_inputs(self, in_maps):
        return [np.concatenate([np.asarray(m[name]) for m in in_maps], axis=0)
                for name in self.in_names]

    def zeros(self):
        return [np.zeros((NCORES * s[0], *s[1:]), d) for (s, d) in self.zero_shapes]

    def run(self, concat_in):
        outs = self.sharded(*concat_in, *self.zeros())
        outs = self.jax.block_until_ready(outs)
        return outs

    def device_inputs(self, concat_in):
        from jax.sharding import NamedSharding
        sh = NamedSharding(self.mesh, self.pspec)
        return [self.jax.device_put(a, sh) for a in concat_in]

    def device_zeros(self):
        import jax.numpy as jnp
        from jax.sharding import NamedSharding
        sh = NamedSharding(self.mesh, self.pspec)
        return [jnp.zeros((NCORES * s[0], *s[1:]), d, device=sh)
                for (s, d) in self.zero_shapes]

    def run_device(self, dev_in):
        outs = self.sharded(*dev_in, *self.device_zeros())
        outs = self.jax.block_until_ready(outs)
        return outs

    def split_out(self, outs):
        res = {}
        for i, name in enumerate(self.out_names):
            res[name] = np.asarray(outs[i]).reshape(NCORES, *self.out_avals[i].shape)
        return res


_CACHE = {}


def _get_runner(repeat=1, loop_n=0, parts=("a", "bc", "d"), pre_parts=()):
    key = ("runner", repeat, loop_n, tuple(parts), tuple(pre_parts))
    if key not in _CACHE:
        nc = build_module(repeat=repeat, loop_n=loop_n, parts=parts, pre_parts=pre_parts)
        _CACHE[key] = _Runner(nc)
    return _CACHE[key]


def kernel(x, W_qkv, b_qkv, W_proj, b_proj):
    runner = _get_runner(repeat=1)
    in_maps = _host_prep(x, W_qkv, b_qkv, W_proj)
    concat_in = runner.concat_inputs(in_maps)
    outs = runner.run(concat_in)
    parts = runner.split_out(outs)["out"]  # [8, 4096, 1024]
    full = parts.sum(axis=0, dtype=np.float64).astype(np.float32)
    full = full + np.asarray(b_proj, np.float32)[None, :]
    return full.reshape(B, T, DM)



# revision 13
# speedup vs baseline: 4.6385x; 4.6385x over previous
"""Causal self-attention (b=2, t=2048, d_model=1024, 16 heads) on 8 trn2 cores.

Sharding: batch x head-quad. Core c handles batch c//4 and the 4 heads
starting at (c%4)*4, as two head-pairs (streams). Each core computes
qkv = x[b] @ W slices, attention for its 4 heads, and a partial
out @ W_proj[head-rows, :] for its batch's 2048 rows. The host sums the 4
partials per batch (the all-reduce after proj) and adds b_proj.

Device pipeline (single phase, fully interleaved):
- xt (d_model-major x) is DMA'd per 512-row chunk on the Pool queue.
- Stage A produces qT/kT per head-pair ([128,rows], 2 heads stacked on
  partitions) and V directly in [rows, head-dims] layout (no PE transposes);
  a ones column per head-block makes the att@V matmul accumulate the softmax
  denominator row. Bias adds ride the PSUM drains (DVE tensor_scalar) and a
  rank-1 ones x bias matmul for V.
- Scores are computed transposed (sT[k,q], both heads of a pair side by
  side in one 2-bank PSUM tile), exp'd on ACT (no max subtraction; logits
  ~N(0,1)), diagonal tiles fixed with one triangular mask multiply.
- Softmax normalization: reciprocal of the accumulated denominator row,
  partition-broadcast via a stride-0 SBUF->SBUF DMA, one DVE multiply per
  head writing the proj-ready ot layout.
- Proj per 128-row tile; PSUM drained to bf16 and DMA'd out (partial sums,
  host reduces).
- Emission interleaves the two streams' k-tiles and weaves stage-A/proj
  pieces into the shared 2-buf PSUM ring so PE/ACT/DVE overlap; exp on ACT
  is the steady-state pacer.
- Optional fp8e4m3 DoubleRow matmuls (2x PE) for stage A and/or proj.
"""

import os
import sys

sys.path.insert(0, "/opt/trn_rl_repo")

import numpy as np

import concourse.bass as bass  # noqa: F401
import concourse.tile as tile
from concourse import bacc, mybir

F32 = mybir.dt.float32
F32R = mybir.dt.float32r
BF16 = mybir.dt.bfloat16
FP8 = mybir.dt.float8e4
EXP = mybir.ActivationFunctionType.Exp
DR = mybir.MatmulPerfMode.DoubleRow

STAGE_A_FP8 = os.environ.get("K_STAGE_A_FP8", "0") == "1"
PROJ_FP8 = os.environ.get("K_PROJ_FP8", "0") == "1"

ADT = BF16                      # attention operands (qt/kt/v/ea/triu)
XDT = FP8 if STAGE_A_FP8 else BF16   # stage A operands (xt, wq/wk/wv)
PDT = FP8 if PROJ_FP8 else BF16      # proj operands (ot, wp)

B = 2
T = 2048
DM = 1024
NH = 16
HD = 64
NCORES = 8
HPC = 4                 # heads per core
NPAIR = 2               # head pairs (streams) per core
ROWS = T                # 2048 rows per core (one batch)
QCH = 512               # query chunk
KTILE = 128             # key tile
NQC = ROWS // QCH       # 4 query chunks
NRT = ROWS // KTILE     # 16 row tiles
NKD = DM // 128         # 8 d_model k-tiles
VW = 2 * (HD + 1)       # 130: v block width per pair (2 heads x (64+ones))


class _Alloc:
    """Tag-based routing to the right tile pool."""
    WORK_BUFS = {"ea": 4, "rc2": 2, "bc": 2, "osb": 3}

    def __init__(self, pers, work, ps):
        self.pers, self.work, self.ps = pers, work, ps

    def tile(self, shape, dt, tag):
        if tag == "ps":
            return self.ps.tile(shape, dt, tag=tag, name=tag, bufs=2)
        if tag == "pso":
            return self.ps.tile(shape, dt, tag=tag, name=tag, bufs=2)
        if tag in self.WORK_BUFS:
            return self.work.tile(shape, dt, tag=tag, name=tag,
                                  bufs=self.WORK_BUFS[tag])
        return self.pers.tile(shape, dt, tag=tag, name=tag)


def _emit_consts(nc, al, aps):
    (xt_d, wq01_d, wq23_d, wk01_d, wk23_d, wv_d, wp_d,
     bq01_d, bq23_d, bk01_d, bk23_d, bvrow_d, triu2_d, out_d) = aps
    C = {"out_d": out_d, "xt_d": xt_d}
    C["xts"] = al.tile([128, NKD * ROWS], XDT, tag="xts")
    C["qtB"] = al.tile([128, NPAIR * ROWS], ADT, tag="qtB")
    C["ktB"] = al.tile([128, NPAIR * ROWS], ADT, tag="ktB")
    C["v0"] = al.tile([128, NRT * VW], ADT, tag="v0")
    C["v1"] = al.tile([128, NRT * VW], ADT, tag="v1")
    C["ot"] = al.tile([128, NPAIR * ROWS], PDT, tag="ot")
    C["wq01"] = al.tile([128, NKD * 128], XDT, tag="wq01")
    C["wq23"] = al.tile([128, NKD * 128], XDT, tag="wq23")
    C["wk01"] = al.tile([128, NKD * 128], XDT, tag="wk01")
    C["wk23"] = al.tile([128, NKD * 128], XDT, tag="wk23")
    C["wv"] = al.tile([128, NKD * 256], XDT, tag="wv")
    C["wp"] = al.tile([128, NPAIR * DM], PDT, tag="wp")
    C["bq01"] = al.tile([128, 1], F32, tag="bq01")
    C["bq23"] = al.tile([128, 1], F32, tag="bq23")
    C["bk01"] = al.tile([128, 1], F32, tag="bk01")
    C["bk23"] = al.tile([128, 1], F32, tag="bk23")
    C["bvrow"] = al.tile([1, 256], F32R, tag="bvrow")
    C["ones1"] = al.tile([1, 128], F32, tag="ones1")
    C["triu2"] = al.tile([128, 256], ADT, tag="triu2")

    for name, dst in (("wq01", wq01_d), ("wq23", wq23_d), ("wk01", wk01_d),
                      ("wk23", wk23_d), ("wv", wv_d), ("wp", wp_d),
                      ("bq01", bq01_d), ("bq23", bq23_d), ("bk01", bk01_d),
                      ("bk23", bk23_d), ("bvrow", bvrow_d), ("triu2", triu2_d)):
        nc.sync.dma_start(C[name][:], dst[:])
    nc.vector.memset(C["ones1"][:], 1.0)
    # ones columns of the V blocks (col 64 and 129 of each 130-block)
    for vp in ("v0", "v1"):
        vb = C[vp].rearrange("p (i w) -> p i w", w=VW)
        nc.vector.memset(vb[:, :, HD], 1.0)
        nc.vector.memset(vb[:, :, 2 * HD + 1], 1.0)
    # pre-loop xt load (iterations 2+ use the in-body prefetches instead)
    xts_v = C["xts"].rearrange("p (k r) -> p k r", k=NKD)
    xt_dv = xt_d.rearrange("p (k r) -> p k r", k=NKD)
    for rc in range(NQC):
        nc.gpsimd.dma_start(xts_v[:, :, rc * QCH:(rc + 1) * QCH],
                            xt_dv[:, :, rc * QCH:(rc + 1) * QCH])
    return C


def _emit_body(nc, al, C, pre=False):
    xts = C["xts"].rearrange("p (k r) -> p k r", k=NKD)
    xt_dv = C["xt_d"].rearrange("p (k r) -> p k r", k=NKD)
    qtb = C["qtB"].rearrange("p (pair r) -> p pair r", pair=NPAIR)
    ktb = C["ktB"].rearrange("p (pair r) -> p pair r", pair=NPAIR)
    otv = C["ot"].rearrange("p (pair r) -> p pair r", pair=NPAIR)
    wpv = C["wp"].rearrange("p (pair o) -> p pair o", pair=NPAIR)
    wvv = C["wv"].rearrange("p (k c) -> p k c", k=NKD)
    triu2v = C["triu2"].rearrange("p (h q) -> p h q", h=2)
    vblk = [C["v0"].rearrange("p (i w) -> p i w", w=VW),
            C["v1"].rearrange("p (i w) -> p i w", w=VW)]

    def xt_load(rc):
        nc.gpsimd.dma_start(xts[:, :, rc * QCH:(rc + 1) * QCH],
                            xt_dv[:, :, rc * QCH:(rc + 1) * QCH])

    def wview(t):
        return t.rearrange("p (k c) -> p k c", k=NKD)

    def a_qk(rc, which, tag="ps"):
        w01, w23 = (C["wq01"], C["wq23"]) if which == "q" else (C["wk01"], C["wk23"])
        b01, b23 = (C["bq01"], C["bq23"]) if which == "q" else (C["bk01"], C["bk23"])
        dst = qtb if which == "q" else ktb
        ps = al.tile([128, 2 * QCH], F32, tag=tag)
        cs = rc * QCH
        if STAGE_A_FP8:
            for kp in range(NKD // 2):
                xr = xts[:, 2 * kp:2 * kp + 2, cs:cs + QCH]
                st, sp = kp == 0, kp == NKD // 2 - 1
                nc.tensor.matmul(ps[:, 0:QCH], wview(w01)[:, 2 * kp:2 * kp + 2, :],
                                 xr, start=st, stop=sp, perf_mode=DR)
                nc.tensor.matmul(ps[:, QCH:], wview(w23)[:, 2 * kp:2 * kp + 2, :],
                                 xr, start=st, stop=sp, perf_mode=DR)
        else:
            for k in range(NKD):
                xr = xts[:, k, cs:cs + QCH]
                st, sp = k == 0, k == NKD - 1
                nc.tensor.matmul(ps[:, 0:QCH], w01[:, k * 128:(k + 1) * 128],
                                 xr, start=st, stop=sp)
                nc.tensor.matmul(ps[:, QCH:], w23[:, k * 128:(k + 1) * 128],
                                 xr, start=st, stop=sp)
        nc.vector.tensor_scalar_add(dst[:, 0, cs:cs + QCH], ps[:, 0:QCH], b01[:])
        nc.vector.tensor_scalar_add(dst[:, 1, cs:cs + QCH], ps[:, QCH:], b23[:])

    def a_v(rc, h):
        ps = al.tile([128, 2 * QCH], F32, tag="ps")
        for j in range(2):
            rt = rc * 4 + h * 2 + j
            base = j * QCH
            if STAGE_A_FP8:
                for kp in range(NKD // 2):
                    xl = xts[:, 2 * kp:2 * kp + 2, rt * 128:(rt + 1) * 128]
                    nc.tensor.matmul(ps[:, base:base + 256],
                                     xl, wvv[:, 2 * kp:2 * kp + 2, :],
                                     start=kp == 0, stop=False, perf_mode=DR)
            else:
                for k in range(NKD):
                    nc.tensor.matmul(ps[:, base:base + 256],
                                     xts[:, k, rt * 128:(rt + 1) * 128],
                                     wvv[:, k, :], start=k == 0, stop=False)
            nc.tensor.matmul(ps[:, base:base + 256],
                             C["ones1"][:, 0:128].bitcast(F32R),
                             C["bvrow"][:], start=False, stop=True)
        # drain both row tiles for each pair: src [p, 2rt, 2head, 64]
        pv = ps.rearrange("p (j x) -> p j x", j=2)
        for pair in range(2):
            src = pv[:, :, pair * 128:(pair + 1) * 128].rearrange(
                "p j (h c) -> p j h c", h=2)
            dstb = vblk[pair][:, rc * 4 + h * 2: rc * 4 + h * 2 + 2, :].rearrange(
                "p j (h w) -> p j h w", h=2)[:, :, :, 0:HD]
            nc.vector.tensor_copy(dstb, src)

    def proj(qc, j):
        q0 = qc * QCH + j * 128
        psp = al.tile([128, 2 * QCH], F32, tag="ps")
        if PROJ_FP8:
            for ct in range(2):
                nc.tensor.matmul(psp[:, ct * QCH:(ct + 1) * QCH],
                                 otv[:, :, q0:q0 + 128],
                                 wpv[:, :, ct * QCH:(ct + 1) * QCH], perf_mode=DR)
        else:
            for ct in range(2):
                for pair in range(2):
                    nc.tensor.matmul(psp[:, ct * QCH:(ct + 1) * QCH],
                                     otv[:, pair, q0:q0 + 128],
                                     wpv[:, pair, ct * QCH:(ct + 1) * QCH],
                                     start=pair == 0, stop=pair == 1)
        osb = al.tile([128, DM], BF16, tag="osb")
        nc.vector.tensor_copy(osb[:], psp[:])
        nc.sync.dma_start(C["out_d"][q0:q0 + 128, :], osb[:])

    def attn_scores(s, qc, kt):
        r = kt * KTILE - qc * QCH
        s0 = max(0, r)
        kcol = kt * KTILE
        qlo = qc * QCH
        ps = al.tile([128, 2 * QCH], F32, tag="ps")
        nc.tensor.matmul(ps[:, s0:QCH], ktb[0:HD, s, kcol:kcol + KTILE],
                         qtb[0:HD, s, qlo + s0:qlo + QCH])
        nc.tensor.matmul(ps[:, QCH + s0:], ktb[HD:128, s, kcol:kcol + KTILE],
                         qtb[HD:128, s, qlo + s0:qlo + QCH])
        ea = al.tile([128, 2 * QCH], ADT, tag="ea")
        pv = ps.rearrange("p (h q) -> p h q", h=2)[:, :, s0:]
        ev = ea.rearrange("p (h q) -> p h q", h=2)[:, :, s0:]
        nc.scalar.activation(ev, pv, EXP, scale=0.125)
        if r >= 0:  # diagonal tile: triangular mask on the 128-col bands
            band = ea.rearrange("p (h q) -> p h q", h=2)[:, :, s0:s0 + KTILE]
            nc.vector.tensor_mul(band, band, triu2v)
        return ea, s0

    def attn_av(s, qc, kt, ea, s0, pso):
        st = kt == 0
        sp = kt == (qc + 1) * 4 - 1
        nc.tensor.matmul(pso[:, s0:QCH], vblk[s][:, kt, 0:HD + 1],
                         ea[:, s0:QCH], start=st, stop=sp)
        nc.tensor.matmul(pso[:, QCH + s0:], vblk[s][:, kt, HD + 1:VW],
                         ea[:, QCH + s0:], start=st, stop=sp)

    def fin_recip(s, pso):
        rc2 = al.tile([1, 2 * QCH], F32, tag="rc2")
        nc.vector.reciprocal(rc2[:], pso[HD:HD + 1, :])
        return rc2

    def fin_bcast(s, rc2):
        bc = al.tile([HD, 2 * QCH], F32, tag="bc")
        nc.gpsimd.partition_broadcast(bc[:], rc2[:])
        return bc

    def fin_muls(s, qc, pso, bc):
        qlo = qc * QCH
        nc.vector.tensor_mul(otv[0:HD, s, qlo:qlo + QCH], pso[0:HD, 0:QCH],
                             bc[:, 0:QCH])
        nc.vector.tensor_mul(otv[HD:128, s, qlo:qlo + QCH], pso[0:HD, QCH:],
                             bc[:, QCH:])

    if pre:
        # pre-loop: stage A rc0 for the first iteration (later iterations
        # get rc0 from the previous body's round 3 + tail)
        a_qk(0, "q")
        a_qk(0, "k")
        a_v(0, 0)
        a_v(0, 1)
        return

    # ---- rounds (stage A rc0 was emitted pre-loop / by the previous
    # iteration's round 3 + tail) ----
    for r in range(NQC):
        nkt = (r + 1) * 4
        aux = []
        if r == 2:  # prefetch next iteration's xt (rc0/rc1 readers are done)
            aux.append(lambda: xt_load(0))
            aux.append(lambda: xt_load(1))
        if r == 3:
            aux.append(lambda: xt_load(2))
            aux.append(lambda: xt_load(3))
        if r + 1 < NQC:
            rc = r + 1
            aux.append(lambda rc=rc: a_qk(rc, "q"))
            aux.append(lambda rc=rc: a_qk(rc, "k"))
            aux.append(lambda rc=rc: a_v(rc, 0))
            aux.append(lambda rc=rc: a_v(rc, 1))
        if r >= 1:
            for j in range(4):
                aux.append(lambda qc=r - 1, j=j: proj(qc, j))
        if r == 3:
            # next iteration's qT/kT rc0; lands late (i>=~10) so this
            # iteration's kt0..3 reads of qtB/ktB are already in the PE past
            aux.append(lambda: a_qk(0, "q"))
            aux.append(lambda: a_qk(0, "k"))
        pso = [al.tile([HD + 1, 2 * QCH], F32, tag="pso") for _ in range(2)]
        prev = [None, None]
        naux = len(aux)
        emitted = 0
        for i in range(nkt):
            for s in range(2):
                if prev[s] is not None:
                    attn_av(s, r, i - 1, *prev[s], pso[s])
                prev[s] = attn_scores(s, r, i)
            while emitted < naux * (i + 1) // nkt:
                aux[emitted]()
                emitted += 1
        for s in range(2):
            attn_av(s, r, nkt - 1, *prev[s], pso[s])
        rcs = [fin_recip(s, pso[s]) for s in range(2)]
        bcs = [fin_bcast(s, rcs[s]) for s in range(2)]
        for s in range(2):
            fin_muls(s, r, pso[s], bcs[s])

    # ---- tail: next iteration's V rc0, proj of last chunk ----
    a_v(0, 0)
    a_v(0, 1)
    for j in range(4):
        proj(NQC - 1, j)


def build_module(repeat=1, loop_n=0, parts=None, pre_parts=()):
    nc = bacc.Bacc("TRN2", target_bir_lowering=False, debug=False,
                   enable_asserts=True, num_devices=NCORES)

    def din(name, shape, dt):
        return nc.dram_tensor(name, shape, dt, kind="ExternalInput").ap()

    aps = (
        din("xt", [128, NKD * ROWS], XDT),
        din("wq01", [128, NKD * 128], XDT),
        din("wq23", [128, NKD * 128], XDT),
        din("wk01", [128, NKD * 128], XDT),
        din("wk23", [128, NKD * 128], XDT),
        din("wv", [128, NKD * 256], XDT),
        din("wp", [128, NPAIR * DM], PDT),
        din("bq01", [128, 1], F32),
        din("bq23", [128, 1], F32),
        din("bk01", [128, 1], F32),
        din("bk23", [128, 1], F32),
        din("bvrow", [1, 256], F32R),
        din("triu2", [128, 256], ADT),
        nc.dram_tensor("out", [ROWS, DM], BF16, kind="ExternalOutput").ap(),
    )
    with tile.TileContext(nc) as tc:
        with tc.tile_pool(name="pers", bufs=1) as pers, \
             tc.tile_pool(name="work", bufs=4) as work, \
             tc.tile_pool(name="ps", bufs=2, space="PSUM") as psp:
            al = _Alloc(pers, work, psp)
            al.tc = tc
            consts = _emit_consts(nc, al, aps)
            _emit_body(nc, al, consts, pre=True)
            if loop_n:
                with tc.For_i(0, loop_n, 1):
                    _emit_body(nc, al, consts)
            else:
                for _ in range(repeat):
                    _emit_body(nc, al, consts)
    nc.compile()
    return nc


def _np_dt(dt):
    import ml_dtypes
    return {BF16: ml_dtypes.bfloat16, FP8: ml_dtypes.float8_e4m3,
            F32: np.float32, F32R: np.float32}[dt]


def _ktile_major(w, ncols):
    """[DM, ncols] -> [128, NKD*ncols] with w[kt*128+p, c] at [p, kt*ncols+c]."""
    return np.ascontiguousarray(
        w.reshape(NKD, 128, ncols).transpose(1, 0, 2).reshape(128, NKD * ncols))


def _host_prep(x, W_qkv, b_qkv, W_proj):
    x = np.asarray(x, np.float32)
    W_qkv = np.asarray(W_qkv, np.float32)
    b_qkv = np.asarray(b_qkv, np.float32)
    W_proj = np.asarray(W_proj, np.float32)
    xdt = _np_dt(XDT)
    adt = _np_dt(ADT)
    pdt = _np_dt(PDT)
    triu = np.triu(np.ones((128, 128), np.float32))
    triu2 = np.concatenate([triu, triu], axis=1).astype(adt)
    in_maps = []
    for c in range(NCORES):
        b = c // 4
        h0 = (c % 4) * 4
        q0 = h0 * HD          # first q column of the 4 heads
        xt = _ktile_major(np.ascontiguousarray(x[b].T), ROWS)
        in_maps.append({
            "xt": xt.astype(xdt),
            "wq01": _ktile_major(W_qkv[:, q0:q0 + 128], 128).astype(xdt),
            "wq23": _ktile_major(W_qkv[:, q0 + 128:q0 + 256], 128).astype(xdt),
            "wk01": _ktile_major(W_qkv[:, DM + q0:DM + q0 + 128], 128).astype(xdt),
            "wk23": _ktile_major(W_qkv[:, DM + q0 + 128:DM + q0 + 256], 128).astype(xdt),
            "wv": _ktile_major(W_qkv[:, 2 * DM + q0:2 * DM + q0 + 256], 256).astype(xdt),
            "wp": np.ascontiguousarray(
                W_proj[q0:q0 + 256, :].reshape(2, 128, DM).transpose(1, 0, 2)
                .reshape(128, 2 * DM)).astype(pdt),
            "bq01": np.ascontiguousarray(b_qkv[q0:q0 + 128, None]),
            "bq23": np.ascontiguousarray(b_qkv[q0 + 128:q0 + 256, None]),
            "bk01": np.ascontiguousarray(b_qkv[DM + q0:DM + q0 + 128, None]),
            "bk23": np.ascontiguousarray(b_qkv[DM + q0 + 128:DM + q0 + 256, None]),
            "bvrow": np.ascontiguousarray(b_qkv[None, 2 * DM + q0:2 * DM + q0 + 256]),
            "triu2": triu2,
        })
    return in_maps


class _Runner:
    """Compile once, execute many times (mirrors bass2jax.run_bass_via_pjrt)."""

    def __init__(self, nc):
        import jax
        from jax.sharding import Mesh, PartitionSpec
        from jax.experimental.shard_map import shard_map
        from concourse import bass2jax
        from concourse import mybir as _mybir

        bass2jax.install_neuronx_cc_hook()
        self.jax = jax
        in_names, out_names, out_avals, zero_shapes = [], [], [], []
        partition_name = nc.partition_id_tensor.name if nc.partition_id_tensor else None
        for alloc in nc.m.functions[0].allocations:
            if not isinstance(alloc, _mybir.MemoryLocationSet):
                continue
            name = alloc.memorylocations[0].name
            if alloc.kind == "ExternalInput":
                if name != partition_name:
                    in_names.append(name)
            elif alloc.kind == "ExternalOutput":
                shape = tuple(alloc.tensor_shape)
                dtype = _mybir.dt.np(alloc.dtype)
                out_names.append(name)
                out_avals.append(jax.core.ShapedArray(shape, dtype))
                zero_shapes.append((shape, dtype))
        self.in_names = in_names
        self.out_names = out_names
        self.out_avals = out_avals
        self.zero_shapes = zero_shapes
        n_params = len(in_names)
        n_outs = len(out_avals)
        all_in_names = in_names + out_names + ([partition_name] if partition_name else [])

        def _body(*args):
            operands = list(args)
            if partition_name is not None:
                operands.append(bass2jax.partition_id_tensor())
            outs = bass2jax._bass_exec_p.bind(
                *operands,
                out_avals=tuple(out_avals),
                in_names=tuple(all_in_names),
                out_names=tuple(out_names),
                lowering_input_output_aliases=(),
                sim_require_finite=True,
                sim_require_nnan=True,
                nc=nc,
            )
            return tuple(outs)

        devices = jax.devices()[:NCORES]
        mesh = Mesh(np.asarray(devices), ("core",))
        self.mesh = mesh
        self.pspec = PartitionSpec("core")
        in_specs = (PartitionSpec("core"),) * (n_params + n_outs)
        out_specs = (PartitionSpec("core"),) * n_outs
        self.donate = tuple(range(n_params, n_params + n_outs))
        self.sharded = jax.jit(
            shard_map(_body, mesh=mesh, in_specs=in_specs, out_specs=out_specs,
                      check_rep=False),
            donate_argnums=self.donate, keep_unused=True)

    def concat_inputs(self, in_maps):
        return [np.concatenate([np.asarray(m[name]) for m in in_maps], axis=0)
                for name in self.in_names]

    def zeros(self):
        return [np.zeros((NCORES * s[0], *s[1:]), d) for (s, d) in self.zero_shapes]

    def run(self, concat_in):
        outs = self.sharded(*concat_in, *self.zeros())
        outs = self.jax.block_until_ready(outs)
        return outs

    def device_inputs(self, concat_in):
        from jax.sharding import NamedSharding
        sh = NamedSharding(self.mesh, self.pspec)
        return [self.jax.device_put(a, sh) for a in concat_in]

    def device_zeros(self):
        import jax.numpy as jnp
        from jax.sharding import NamedSharding
        sh = NamedSharding(self.mesh, self.pspec)
        return [jnp.zeros((NCORES * s[0], *s[1:]), d, device=sh)
                for (s, d) in self.zero_shapes]

    def run_device(self, dev_in):
        outs = self.sharded(*dev_in, *self.device_zeros())
        outs = self.jax.block_until_ready(outs)
        return outs

    def split_out(self, outs):
        res = {}
        for i, name in enumerate(self.out_names):
            res[name] = np.asarray(outs[i]).reshape(NCORES, *self.out_avals[i].shape)
        return res


_CACHE = {}


def _get_runner(repeat=1, loop_n=0, parts=None, pre_parts=()):
    key = ("runner", repeat, loop_n, STAGE_A_FP8, PROJ_FP8)
    if key not in _CACHE:
        nc = build_module(repeat=repeat, loop_n=loop_n)
        _CACHE[key] = _Runner(nc)
    return _CACHE[key]


def kernel(x, W_qkv, b_qkv, W_proj, b_proj):
    runner = _get_runner(repeat=1)
    in_maps = _host_prep(x, W_qkv, b_qkv, W_proj)
    concat_in = runner.concat_inputs(in_maps)
    outs = runner.run(concat_in)
    parts = runner.split_out(outs)["out"]  # [8, 2048, 1024] bf16
    parts = parts.astype(np.float32)
    b_proj = np.asarray(b_proj, np.float32)
    full = np.stack([parts[4 * b:4 * b + 4].sum(axis=0) + b_proj[None, :]
                     for b in range(B)])
    return full.astype(np.float32)


# revision 21
# speedup vs baseline: 4.8624x; 1.0483x over previous
"""Causal self-attention (b=2, t=2048, d_model=1024, 16 heads) on 8 trn2 cores.

Sharding: batch x head-quad. Core c handles batch c//4 and the 4 heads
starting at (c%4)*4, as two head-pairs (streams). Each core computes
qkv = x[b] @ W slices, attention for its 4 heads, and a partial
out @ W_proj[head-rows, :] for its batch's 2048 rows. The host sums the 4
partials per batch (the all-reduce after proj) and adds b_proj.

Device pipeline (single phase, fully interleaved):
- xt (d_model-major x) is DMA'd per 512-row chunk on the Pool queue.
- Stage A produces qT/kT per head-pair ([128,rows], 2 heads stacked on
  partitions) and V directly in [rows, head-dims] layout (no PE transposes);
  a ones column per head-block makes the att@V matmul accumulate the softmax
  denominator row. Bias adds ride the PSUM drains (DVE tensor_scalar) and a
  rank-1 ones x bias matmul for V.
- Scores are computed transposed (sT[k,q], both heads of a pair side by
  side in one 2-bank PSUM tile), exp'd on ACT (no max subtraction; logits
  ~N(0,1)), diagonal tiles fixed with one triangular mask multiply.
- Softmax normalization: reciprocal of the accumulated denominator row,
  partition-broadcast via a stride-0 SBUF->SBUF DMA, one DVE multiply per
  head writing the proj-ready ot layout.
- Proj per 128-row tile; PSUM drained to bf16 and DMA'd out (partial sums,
  host reduces).
- Emission interleaves the two streams' k-tiles and weaves stage-A/proj
  pieces into the shared 2-buf PSUM ring so PE/ACT/DVE overlap; exp on ACT
  is the steady-state pacer.
- Optional fp8e4m3 DoubleRow matmuls (2x PE) for stage A and/or proj.
"""

import os
import sys

sys.path.insert(0, "/opt/trn_rl_repo")

import numpy as np

import concourse.bass as bass  # noqa: F401
import concourse.tile as tile
from concourse import bacc, mybir

F32 = mybir.dt.float32
F32R = mybir.dt.float32r
BF16 = mybir.dt.bfloat16
FP8 = mybir.dt.float8e4
EXP = mybir.ActivationFunctionType.Exp
DR = mybir.MatmulPerfMode.DoubleRow

STAGE_A_FP8 = os.environ.get("K_STAGE_A_FP8", "0") == "1"
PROJ_FP8 = os.environ.get("K_PROJ_FP8", "0") == "1"
NO_AV = os.environ.get("K_NO_AV", "0") == "1"      # diagnostic: skip AV matmuls
NO_EXP = os.environ.get("K_NO_EXP", "0") == "1"    # diagnostic: skip exp/mask
SCORE_PRI = int(os.environ.get("K_SCORE_PRI", "0"))  # score matmul priority boost

ADT = BF16                      # attention operands (qt/kt/v/ea/triu)
XDT = FP8 if STAGE_A_FP8 else BF16   # stage A operands (xt, wq/wk/wv)
PDT = FP8 if PROJ_FP8 else BF16      # proj operands (ot, wp)

B = 2
T = 2048
DM = 1024
NH = 16
HD = 64
NCORES = 8
HPC = 4                 # heads per core
NPAIR = 2               # head pairs (streams) per core
ROWS = T                # 2048 rows per core (one batch)
QCH = 512               # query chunk
KTILE = 128             # key tile
NQC = ROWS // QCH       # 4 query chunks
NRT = ROWS // KTILE     # 16 row tiles
NKD = DM // 128         # 8 d_model k-tiles
VW = 2 * (HD + 1)       # 130: v block width per pair (2 heads x (64+ones))


class _Alloc:
    """Tag-based routing to the right tile pool."""
    WORK_BUFS = {"ea": 6, "rc2": 3, "bc": 3, "osb": 4}

    def __init__(self, pers, work, ps):
        self.pers, self.work, self.ps = pers, work, ps

    def tile(self, shape, dt, tag):
        if tag == "ps":
            return self.ps.tile(shape, dt, tag=tag, name=tag, bufs=2)
        if tag == "pso":
            return self.ps.tile(shape, dt, tag=tag, name=tag, bufs=2)
        if tag in self.WORK_BUFS:
            return self.work.tile(shape, dt, tag=tag, name=tag,
                                  bufs=self.WORK_BUFS[tag])
        return self.pers.tile(shape, dt, tag=tag, name=tag)


def _emit_consts(nc, al, aps):
    (xt_d, wq01_d, wq23_d, wk01_d, wk23_d, wv_d, wp_d,
     bq01_d, bq23_d, bk01_d, bk23_d, bvrow_d, triu2_d, out_d) = aps
    C = {"out_d": out_d, "xt_d": xt_d}
    C["xts"] = al.tile([128, NKD * ROWS], XDT, tag="xts")
    C["qtB"] = al.tile([128, NPAIR * ROWS], ADT, tag="qtB")
    C["ktB"] = al.tile([128, NPAIR * ROWS], ADT, tag="ktB")
    C["v0"] = al.tile([128, NRT * VW], ADT, tag="v0")
    C["v1"] = al.tile([128, NRT * VW], ADT, tag="v1")
    C["ot"] = al.tile([128, NPAIR * ROWS], PDT, tag="ot")
    C["wq01"] = al.tile([128, NKD * 128], XDT, tag="wq01")
    C["wq23"] = al.tile([128, NKD * 128], XDT, tag="wq23")
    C["wk01"] = al.tile([128, NKD * 128], XDT, tag="wk01")
    C["wk23"] = al.tile([128, NKD * 128], XDT, tag="wk23")
    C["wv"] = al.tile([128, NKD * 256], XDT, tag="wv")
    C["wp"] = al.tile([128, NPAIR * DM], PDT, tag="wp")
    C["bq01"] = al.tile([128, 1], F32, tag="bq01")
    C["bq23"] = al.tile([128, 1], F32, tag="bq23")
    C["bk01"] = al.tile([128, 1], F32, tag="bk01")
    C["bk23"] = al.tile([128, 1], F32, tag="bk23")
    C["bvrow"] = al.tile([1, 256], F32R, tag="bvrow")
    C["ones1"] = al.tile([1, 128], F32, tag="ones1")
    C["triu2"] = al.tile([128, 256], ADT, tag="triu2")

    for name, dst in (("wq01", wq01_d), ("wq23", wq23_d), ("wk01", wk01_d),
                      ("wk23", wk23_d), ("wv", wv_d), ("wp", wp_d),
                      ("bq01", bq01_d), ("bq23", bq23_d), ("bk01", bk01_d),
                      ("bk23", bk23_d), ("bvrow", bvrow_d), ("triu2", triu2_d)):
        nc.sync.dma_start(C[name][:], dst[:])
    nc.vector.memset(C["ones1"][:], 1.0)
    # ones columns of the V blocks (col 64 and 129 of each 130-block)
    for vp in ("v0", "v1"):
        vb = C[vp].rearrange("p (i w) -> p i w", w=VW)
        nc.vector.memset(vb[:, :, HD], 1.0)
        nc.vector.memset(vb[:, :, 2 * HD + 1], 1.0)
    # pre-loop xt load (iterations 2+ use the in-body prefetches instead)
    xts_v = C["xts"].rearrange("p (k r) -> p k r", k=NKD)
    xt_dv = xt_d.rearrange("p (k r) -> p k r", k=NKD)
    for rc in range(NQC):
        nc.gpsimd.dma_start(xts_v[:, :, rc * QCH:(rc + 1) * QCH],
                            xt_dv[:, :, rc * QCH:(rc + 1) * QCH])
    return C


def _emit_body(nc, al, C, pre=False, parts=("a", "attn", "proj")):
    xts = C["xts"].rearrange("p (k r) -> p k r", k=NKD)
    xt_dv = C["xt_d"].rearrange("p (k r) -> p k r", k=NKD)
    qtb = C["qtB"].rearrange("p (pair r) -> p pair r", pair=NPAIR)
    ktb = C["ktB"].rearrange("p (pair r) -> p pair r", pair=NPAIR)
    otv = C["ot"].rearrange("p (pair r) -> p pair r", pair=NPAIR)
    wpv = C["wp"].rearrange("p (pair o) -> p pair o", pair=NPAIR)
    wvv = C["wv"].rearrange("p (k c) -> p k c", k=NKD)
    triu2v = C["triu2"].rearrange("p (h q) -> p h q", h=2)
    vblk = [C["v0"].rearrange("p (i w) -> p i w", w=VW),
            C["v1"].rearrange("p (i w) -> p i w", w=VW)]

    def xt_load(rc):
        nc.gpsimd.dma_start(xts[:, :, rc * QCH:(rc + 1) * QCH],
                            xt_dv[:, :, rc * QCH:(rc + 1) * QCH])

    def wview(t):
        return t.rearrange("p (k c) -> p k c", k=NKD)

    def a_qk(rc, which, tag="ps"):
        w01, w23 = (C["wq01"], C["wq23"]) if which == "q" else (C["wk01"], C["wk23"])
        b01, b23 = (C["bq01"], C["bq23"]) if which == "q" else (C["bk01"], C["bk23"])
        dst = qtb if which == "q" else ktb
        ps = al.tile([128, 2 * QCH], F32, tag=tag)
        cs = rc * QCH
        if STAGE_A_FP8:
            for kp in range(NKD // 2):
                xr = xts[:, 2 * kp:2 * kp + 2, cs:cs + QCH]
                st, sp = kp == 0, kp == NKD // 2 - 1
                nc.tensor.matmul(ps[:, 0:QCH], wview(w01)[:, 2 * kp:2 * kp + 2, :],
                                 xr, start=st, stop=sp, perf_mode=DR)
                nc.tensor.matmul(ps[:, QCH:], wview(w23)[:, 2 * kp:2 * kp + 2, :],
                                 xr, start=st, stop=sp, perf_mode=DR)
        else:
            for k in range(NKD):
                xr = xts[:, k, cs:cs + QCH]
                st, sp = k == 0, k == NKD - 1
                nc.tensor.matmul(ps[:, 0:QCH], w01[:, k * 128:(k + 1) * 128],
                                 xr, start=st, stop=sp)
                nc.tensor.matmul(ps[:, QCH:], w23[:, k * 128:(k + 1) * 128],
                                 xr, start=st, stop=sp)
        nc.vector.tensor_scalar_add(dst[:, 0, cs:cs + QCH], ps[:, 0:QCH], b01[:])
        nc.vector.tensor_scalar_add(dst[:, 1, cs:cs + QCH], ps[:, QCH:], b23[:])

    def a_v(rc, h):
        ps = al.tile([128, 2 * QCH], F32, tag="ps")
        for j in range(2):
            rt = rc * 4 + h * 2 + j
            base = j * QCH
            if STAGE_A_FP8:
                for kp in range(NKD // 2):
                    xl = xts[:, 2 * kp:2 * kp + 2, rt * 128:(rt + 1) * 128]
                    nc.tensor.matmul(ps[:, base:base + 256],
                                     xl, wvv[:, 2 * kp:2 * kp + 2, :],
                                     start=kp == 0, stop=False, perf_mode=DR)
            else:
                for k in range(NKD):
                    nc.tensor.matmul(ps[:, base:base + 256],
                                     xts[:, k, rt * 128:(rt + 1) * 128],
                                     wvv[:, k, :], start=k == 0, stop=False)
            nc.tensor.matmul(ps[:, base:base + 256],
                             C["ones1"][:, 0:128].bitcast(F32R),
                             C["bvrow"][:], start=False, stop=True)
        # drain both row tiles for each pair: src [p, 2rt, 2head, 64]
        pv = ps.rearrange("p (j x) -> p j x", j=2)
        for pair in range(2):
            src = pv[:, :, pair * 128:(pair + 1) * 128].rearrange(
                "p j (h c) -> p j h c", h=2)
            dstb = vblk[pair][:, rc * 4 + h * 2: rc * 4 + h * 2 + 2, :].rearrange(
                "p j (h w) -> p j h w", h=2)[:, :, :, 0:HD]
            nc.vector.tensor_copy(dstb, src)

    def proj(qc, j):
        q0 = qc * QCH + j * 128
        psp = al.tile([128, 2 * QCH], F32, tag="ps")
        if PROJ_FP8:
            for ct in range(2):
                nc.tensor.matmul(psp[:, ct * QCH:(ct + 1) * QCH],
                                 otv[:, :, q0:q0 + 128],
                                 wpv[:, :, ct * QCH:(ct + 1) * QCH], perf_mode=DR)
        else:
            # pair-outer so the ot stationary is reused across both ct MMs
            for pair in range(2):
                for ct in range(2):
                    nc.tensor.matmul(psp[:, ct * QCH:(ct + 1) * QCH],
                                     otv[:, pair, q0:q0 + 128],
                                     wpv[:, pair, ct * QCH:(ct + 1) * QCH],
                                     start=pair == 0, stop=pair == 1)
        osb = al.tile([128, DM], BF16, tag="osb")
        nc.vector.tensor_copy(osb[:], psp[:])
        nc.sync.dma_start(C["out_d"][q0:q0 + 128, :], osb[:])

    def attn_scores(s, qc, kt):
        r = kt * KTILE - qc * QCH
        s0 = max(0, r)
        kcol = kt * KTILE
        qlo = qc * QCH
        ps = al.tile([128, 2 * QCH], F32, tag="ps")
        from contextlib import nullcontext
        with (al.tc.high_priority(offset=SCORE_PRI) if SCORE_PRI else nullcontext()):
            nc.tensor.matmul(ps[:, s0:QCH], ktb[0:HD, s, kcol:kcol + KTILE],
                             qtb[0:HD, s, qlo + s0:qlo + QCH])
            nc.tensor.matmul(ps[:, QCH + s0:], ktb[HD:128, s, kcol:kcol + KTILE],
                             qtb[HD:128, s, qlo + s0:qlo + QCH])
        ea = al.tile([128, 2 * QCH], ADT, tag="ea")
        if NO_EXP:
            nc.vector.memset(ea[:, 0:1], 1.0)  # diagnostic: tiny placeholder write
        else:
            pv = ps.rearrange("p (h q) -> p h q", h=2)[:, :, s0:]
            ev = ea.rearrange("p (h q) -> p h q", h=2)[:, :, s0:]
            nc.scalar.activation(ev, pv, EXP, scale=0.125)
            if r >= 0:  # diagonal tile: triangular mask on the 128-col bands
                band = ea.rearrange("p (h q) -> p h q", h=2)[:, :, s0:s0 + KTILE]
                nc.vector.tensor_mul(band, band, triu2v)
        return ea, s0

    def attn_av(s, qc, kt, ea, s0, pso):
        if NO_AV:
            return
        st = kt == 0
        sp = kt == (qc + 1) * 4 - 1
        nc.tensor.matmul(pso[:, s0:QCH], vblk[s][:, kt, 0:HD + 1],
                         ea[:, s0:QCH], start=st, stop=sp)
        nc.tensor.matmul(pso[:, QCH + s0:], vblk[s][:, kt, HD + 1:VW],
                         ea[:, QCH + s0:], start=st, stop=sp)

    def fin_recip(s, pso):
        rc2 = al.tile([1, 2 * QCH], F32, tag="rc2")
        nc.vector.reciprocal(rc2[:], pso[HD:HD + 1, :])
        return rc2

    def fin_bcast(s, rc2):
        bc = al.tile([HD, 2 * QCH], F32, tag="bc")
        nc.gpsimd.partition_broadcast(bc[:], rc2[:])
        return bc

    def fin_muls(s, qc, pso, bc):
        qlo = qc * QCH
        nc.vector.tensor_mul(otv[0:HD, s, qlo:qlo + QCH], pso[0:HD, 0:QCH],
                             bc[:, 0:QCH])
        nc.vector.tensor_mul(otv[HD:128, s, qlo:qlo + QCH], pso[0:HD, QCH:],
                             bc[:, QCH:])

    if pre:
        # pre-loop: stage A rc0 for the first iteration (later iterations
        # get rc0 from the previous body's round 3 + tail)
        if "a" in parts:
            a_qk(0, "q")
            a_qk(0, "k")
            a_v(0, 0)
            a_v(0, 1)
        return

    # ---- rounds (stage A rc0 was emitted pre-loop / by the previous
    # iteration's round 3 + tail) ----
    for r in range(NQC):
        nkt = (r + 1) * 4
        aux = []
        if "a" in parts:
            if r == 2:  # prefetch next iteration's xt (rc0/rc1 readers done)
                aux.append(lambda: xt_load(0))
                aux.append(lambda: xt_load(1))
            if r == 3:
                aux.append(lambda: xt_load(2))
                aux.append(lambda: xt_load(3))
            if r + 1 < NQC:
                rc = r + 1
                aux.append(lambda rc=rc: a_qk(rc, "q"))
                aux.append(lambda rc=rc: a_qk(rc, "k"))
                aux.append(lambda rc=rc: a_v(rc, 0))
                aux.append(lambda rc=rc: a_v(rc, 1))
        if "proj" in parts and r >= 1:
            for j in range(4):
                aux.append(lambda qc=r - 1, j=j: proj(qc, j))
        if "a" in parts and r == 3:
            # next iteration's qT/kT rc0; lands late (i>=~10) so this
            # iteration's kt0..3 reads of qtB/ktB are already in the PE past
            aux.append(lambda: a_qk(0, "q"))
            aux.append(lambda: a_qk(0, "k"))
        if "attn" not in parts:
            for fn in aux:
                fn()
            continue
        # sequential streams: stream 0's whole chunk, then stream 1's.
        # The shared 2-buf ring then gives a depth-2 score pipeline within
        # each chunk, and each stream's finalize chain (recip -> broadcast ->
        # muls) hides under the other stream's units.
        naux = len(aux)
        emitted = 0
        total_units = 2 * nkt
        unit = 0
        for s in range(2):
            pso = al.tile([HD + 1, 2 * QCH], F32, tag="pso")
            prev = None
            for i in range(nkt):
                if prev is not None:
                    attn_av(s, r, i - 1, *prev, pso)
                prev = attn_scores(s, r, i)
                unit += 1
                while emitted < naux * unit // total_units:
                    aux[emitted]()
                    emitted += 1
            attn_av(s, r, nkt - 1, *prev, pso)
            if not NO_AV:
                rc2 = fin_recip(s, pso)
                bc = fin_bcast(s, rc2)
                fin_muls(s, r, pso, bc)

    # ---- tail: next iteration's V rc0, proj of last chunk ----
    if "a" in parts:
        a_v(0, 0)
        a_v(0, 1)
    if "proj" in parts:
        for j in range(4):
            proj(NQC - 1, j)


def _maybe_enable_ldw_opt():
    """Flip walrus's --enable-ldw-opt to true (background weight buffer /
    Ldweights pull-ahead) for this process's compiles."""
    if os.environ.get("K_LDW_OPT", "0") != "1":
        return
    from concourse import bass_utils as _bu
    if getattr(_bu, "_k_ldw_patched", False):
        return
    _orig = _bu.run_command

    def _rc(cmd, *a, **k):
        if isinstance(cmd, list):
            cmd = ["--enable-ldw-opt=true" if c == "--enable-ldw-opt=false" else c
                   for c in cmd]
        return _orig(cmd, *a, **k)

    _bu.run_command = _rc
    _bu._k_ldw_patched = True


def build_module(repeat=1, loop_n=0, parts=("a", "attn", "proj"), pre_parts=()):
    _maybe_enable_ldw_opt()
    nc = bacc.Bacc("TRN2", target_bir_lowering=False, debug=False,
                   enable_asserts=True, num_devices=NCORES)

    def din(name, shape, dt):
        return nc.dram_tensor(name, shape, dt, kind="ExternalInput").ap()

    aps = (
        din("xt", [128, NKD * ROWS], XDT),
        din("wq01", [128, NKD * 128], XDT),
        din("wq23", [128, NKD * 128], XDT),
        din("wk01", [128, NKD * 128], XDT),
        din("wk23", [128, NKD * 128], XDT),
        din("wv", [128, NKD * 256], XDT),
        din("wp", [128, NPAIR * DM], PDT),
        din("bq01", [128, 1], F32),
        din("bq23", [128, 1], F32),
        din("bk01", [128, 1], F32),
        din("bk23", [128, 1], F32),
        din("bvrow", [1, 256], F32R),
        din("triu2", [128, 256], ADT),
        nc.dram_tensor("out", [ROWS, DM], BF16, kind="ExternalOutput").ap(),
    )
    with tile.TileContext(nc) as tc:
        with tc.tile_pool(name="pers", bufs=1) as pers, \
             tc.tile_pool(name="work", bufs=4) as work, \
             tc.tile_pool(name="ps", bufs=2, space="PSUM") as psp:
            al = _Alloc(pers, work, psp)
            al.tc = tc
            consts = _emit_consts(nc, al, aps)
            _emit_body(nc, al, consts, pre=True, parts=parts)
            if loop_n:
                with tc.For_i(0, loop_n, 1):
                    _emit_body(nc, al, consts, parts=parts)
            else:
                for _ in range(repeat):
                    _emit_body(nc, al, consts, parts=parts)
    nc.compile()
    return nc


def _np_dt(dt):
    import ml_dtypes
    return {BF16: ml_dtypes.bfloat16, FP8: ml_dtypes.float8_e4m3,
            F32: np.float32, F32R: np.float32}[dt]


def _ktile_major(w, ncols):
    """[DM, ncols] -> [128, NKD*ncols] with w[kt*128+p, c] at [p, kt*ncols+c]."""
    return np.ascontiguousarray(
        w.reshape(NKD, 128, ncols).transpose(1, 0, 2).reshape(128, NKD * ncols))


def _host_prep(x, W_qkv, b_qkv, W_proj):
    x = np.asarray(x, np.float32)
    W_qkv = np.asarray(W_qkv, np.float32)
    b_qkv = np.asarray(b_qkv, np.float32)
    W_proj = np.asarray(W_proj, np.float32)
    xdt = _np_dt(XDT)
    adt = _np_dt(ADT)
    pdt = _np_dt(PDT)
    triu = np.triu(np.ones((128, 128), np.float32))
    triu2 = np.concatenate([triu, triu], axis=1).astype(adt)
    in_maps = []
    for c in range(NCORES):
        b = c // 4
        h0 = (c % 4) * 4
        q0 = h0 * HD          # first q column of the 4 heads
        xt = _ktile_major(np.ascontiguousarray(x[b].T), ROWS)
        in_maps.append({
            "xt": xt.astype(xdt),
            "wq01": _ktile_major(W_qkv[:, q0:q0 + 128], 128).astype(xdt),
            "wq23": _ktile_major(W_qkv[:, q0 + 128:q0 + 256], 128).astype(xdt),
            "wk01": _ktile_major(W_qkv[:, DM + q0:DM + q0 + 128], 128).astype(xdt),
            "wk23": _ktile_major(W_qkv[:, DM + q0 + 128:DM + q0 + 256], 128).astype(xdt),
            "wv": _ktile_major(W_qkv[:, 2 * DM + q0:2 * DM + q0 + 256], 256).astype(xdt),
            "wp": np.ascontiguousarray(
                W_proj[q0:q0 + 256, :].reshape(2, 128, DM).transpose(1, 0, 2)
                .reshape(128, 2 * DM)).astype(pdt),
            "bq01": np.ascontiguousarray(b_qkv[q0:q0 + 128, None]),
            "bq23": np.ascontiguousarray(b_qkv[q0 + 128:q0 + 256, None]),
            "bk01": np.ascontiguousarray(b_qkv[DM + q0:DM + q0 + 128, None]),
            "bk23": np.ascontiguousarray(b_qkv[DM + q0 + 128:DM + q0 + 256, None]),
            "bvrow": np.ascontiguousarray(b_qkv[None, 2 * DM + q0:2 * DM + q0 + 256]),
            "triu2": triu2,
        })
    return in_maps


class _Runner:
    """Compile once, execute many times (mirrors bass2jax.run_bass_via_pjrt)."""

    def __init__(self, nc):
        import jax
        from jax.sharding import Mesh, PartitionSpec
        from jax.experimental.shard_map import shard_map
        from concourse import bass2jax
        from concourse import mybir as _mybir

        bass2jax.install_neuronx_cc_hook()
        self.jax = jax
        in_names, out_names, out_avals, zero_shapes = [], [], [], []
        partition_name = nc.partition_id_tensor.name if nc.partition_id_tensor else None
        for alloc in nc.m.functions[0].allocations:
            if not isinstance(alloc, _mybir.MemoryLocationSet):
                continue
            name = alloc.memorylocations[0].name
            if alloc.kind == "ExternalInput":
                if name != partition_name:
                    in_names.append(name)
            elif alloc.kind == "ExternalOutput":
                shape = tuple(alloc.tensor_shape)
                dtype = _mybir.dt.np(alloc.dtype)
                out_names.append(name)
                out_avals.append(jax.core.ShapedArray(shape, dtype))
                zero_shapes.append((shape, dtype))
        self.in_names = in_names
        self.out_names = out_names
        self.out_avals = out_avals
        self.zero_shapes = zero_shapes
        n_params = len(in_names)
        n_outs = len(out_avals)
        all_in_names = in_names + out_names + ([partition_name] if partition_name else [])

        def _body(*args):
            operands = list(args)
            if partition_name is not None:
                operands.append(bass2jax.partition_id_tensor())
            outs = bass2jax._bass_exec_p.bind(
                *operands,
                out_avals=tuple(out_avals),
                in_names=tuple(all_in_names),
                out_names=tuple(out_names),
                lowering_input_output_aliases=(),
                sim_require_finite=True,
                sim_require_nnan=True,
                nc=nc,
            )
            return tuple(outs)

        devices = jax.devices()[:NCORES]
        mesh = Mesh(np.asarray(devices), ("core",))
        self.mesh = mesh
        self.pspec = PartitionSpec("core")
        in_specs = (PartitionSpec("core"),) * (n_params + n_outs)
        out_specs = (PartitionSpec("core"),) * n_outs
        self.donate = tuple(range(n_params, n_params + n_outs))
        self.sharded = jax.jit(
            shard_map(_body, mesh=mesh, in_specs=in_specs, out_specs=out_specs,
                      check_rep=False),
            donate_argnums=self.donate, keep_unused=True)

    def concat_inputs(self, in_maps):
        return [np.concatenate([np.asarray(m[name]) for m in in_maps], axis=0)
                for name in self.in_names]

    def zeros(self):
        return [np.zeros((NCORES * s[0], *s[1:]), d) for (s, d) in self.zero_shapes]

    def run(self, concat_in):
        outs = self.sharded(*concat_in, *self.zeros())
        outs = self.jax.block_until_ready(outs)
        return outs

    def device_inputs(self, concat_in):
        from jax.sharding import NamedSharding
        sh = NamedSharding(self.mesh, self.pspec)
        return [self.jax.device_put(a, sh) for a in concat_in]

    def device_zeros(self):
        import jax.numpy as jnp
        from jax.sharding import NamedSharding
        sh = NamedSharding(self.mesh, self.pspec)
        return [jnp.zeros((NCORES * s[0], *s[1:]), d, device=sh)
                for (s, d) in self.zero_shapes]

    def run_device(self, dev_in):
        outs = self.sharded(*dev_in, *self.device_zeros())
        outs = self.jax.block_until_ready(outs)
        return outs

    def split_out(self, outs):
        res = {}
        for i, name in enumerate(self.out_names):
            res[name] = np.asarray(outs[i]).reshape(NCORES, *self.out_avals[i].shape)
        return res


_CACHE = {}


def _get_runner(repeat=1, loop_n=0, parts=("a", "attn", "proj"), pre_parts=()):
    parts = tuple(parts) if parts else ("a", "attn", "proj")
    key = ("runner", repeat, loop_n, STAGE_A_FP8, PROJ_FP8, parts)
    if key not in _CACHE:
        nc = build_module(repeat=repeat, loop_n=loop_n, parts=parts)
        _CACHE[key] = _Runner(nc)
    return _CACHE[key]


def kernel(x, W_qkv, b_qkv, W_proj, b_proj):
    runner = _get_runner(repeat=1)
    in_maps = _host_prep(x, W_qkv, b_qkv, W_proj)
    concat_in = runner.concat_inputs(in_maps)
    outs = runner.run(concat_in)
    parts = runner.split_out(outs)["out"]  # [8, 2048, 1024] bf16
    parts = parts.astype(np.float32)
    b_proj = np.asarray(b_proj, np.float32)
    full = np.stack([parts[4 * b:4 * b + 4].sum(axis=0) + b_proj[None, :]
                     for b in range(B)])
    return full.astype(np.float32)


# revision 22
# speedup vs baseline: 5.4027x; 1.1111x over previous
"""Causal self-attention (b=2, t=2048, d_model=1024, 16 heads) on 8 trn2 cores.

Sharding: batch x head-quad. Core c handles batch c//4 and the 4 heads
starting at (c%4)*4, as two head-pairs (streams). Each core computes
qkv = x[b] @ W slices, attention for its 4 heads, and a partial
out @ W_proj[head-rows, :] for its batch's 2048 rows. The host sums the 4
partials per batch (the all-reduce after proj) and adds b_proj.

Device pipeline (single phase, fully interleaved):
- xt (d_model-major x) is DMA'd per 512-row chunk on the Pool queue.
- Stage A produces qT/kT per head-pair ([128,rows], 2 heads stacked on
  partitions) and V directly in [rows, head-dims] layout (no PE transposes);
  a ones column per head-block makes the att@V matmul accumulate the softmax
  denominator row. Bias adds ride the PSUM drains (DVE tensor_scalar) and a
  rank-1 ones x bias matmul for V.
- Scores are computed transposed (sT[k,q], both heads of a pair side by
  side in one 2-bank PSUM tile), exp'd on ACT (no max subtraction; logits
  ~N(0,1)), diagonal tiles fixed with one triangular mask multiply.
- Softmax normalization: reciprocal of the accumulated denominator row,
  partition-broadcast via a stride-0 SBUF->SBUF DMA, one DVE multiply per
  head writing the proj-ready ot layout.
- Proj per 128-row tile; PSUM drained to bf16 and DMA'd out (partial sums,
  host reduces).
- Emission interleaves the two streams' k-tiles and weaves stage-A/proj
  pieces into the shared 2-buf PSUM ring so PE/ACT/DVE overlap; exp on ACT
  is the steady-state pacer.
- Optional fp8e4m3 DoubleRow matmuls (2x PE) for stage A and/or proj.
"""

import os
import sys

sys.path.insert(0, "/opt/trn_rl_repo")

import numpy as np

import concourse.bass as bass  # noqa: F401
import concourse.tile as tile
from concourse import bacc, mybir

F32 = mybir.dt.float32
F32R = mybir.dt.float32r
BF16 = mybir.dt.bfloat16
FP8 = mybir.dt.float8e4
EXP = mybir.ActivationFunctionType.Exp
DR = mybir.MatmulPerfMode.DoubleRow

STAGE_A_FP8 = os.environ.get("K_STAGE_A_FP8", "0") == "1"
PROJ_FP8 = os.environ.get("K_PROJ_FP8", "0") == "1"
NO_AV = os.environ.get("K_NO_AV", "0") == "1"      # diagnostic: skip AV matmuls
NO_EXP = os.environ.get("K_NO_EXP", "0") == "1"    # diagnostic: skip exp/mask
SCORE_PRI = int(os.environ.get("K_SCORE_PRI", "0"))  # score matmul priority boost

ADT = BF16                      # attention operands (qt/kt/v/ea/triu)
XDT = FP8 if STAGE_A_FP8 else BF16   # stage A operands (xt, wq/wk/wv)
PDT = FP8 if PROJ_FP8 else BF16      # proj operands (ot, wp)

B = 2
T = 2048
DM = 1024
NH = 16
HD = 64
NCORES = 8
HPC = 4                 # heads per core
NPAIR = 2               # head pairs (streams) per core
ROWS = T                # 2048 rows per core (one batch)
QCH = 512               # query chunk
KTILE = 128             # key tile
NQC = ROWS // QCH       # 4 query chunks
NRT = ROWS // KTILE     # 16 row tiles
NKD = DM // 128         # 8 d_model k-tiles
VW = 2 * (HD + 1)       # 130: v block width per pair (2 heads x (64+ones))


class _Alloc:
    """Tag-based routing to the right tile pool."""
    WORK_BUFS = {"ea": 6, "rc2": 3, "bc": 3, "osb": 4}

    def __init__(self, pers, work, ps):
        self.pers, self.work, self.ps = pers, work, ps

    def tile(self, shape, dt, tag):
        if tag == "ps":
            return self.ps.tile(shape, dt, tag=tag, name=tag, bufs=2)
        if tag == "pso":
            return self.ps.tile(shape, dt, tag=tag, name=tag, bufs=2)
        if tag in self.WORK_BUFS:
            return self.work.tile(shape, dt, tag=tag, name=tag,
                                  bufs=self.WORK_BUFS[tag])
        return self.pers.tile(shape, dt, tag=tag, name=tag)


def _emit_consts(nc, al, aps):
    (xt_d, wq01_d, wq23_d, wk01_d, wk23_d, wv_d, wp_d,
     bq01_d, bq23_d, bk01_d, bk23_d, bvrow_d, triu2_d, out_d) = aps
    C = {"out_d": out_d, "xt_d": xt_d}
    C["xts"] = al.tile([128, NKD * ROWS], XDT, tag="xts")
    C["qtB"] = al.tile([128, NPAIR * ROWS], ADT, tag="qtB")
    C["ktB"] = al.tile([128, NPAIR * ROWS], ADT, tag="ktB")
    C["v0"] = al.tile([128, NRT * VW], ADT, tag="v0")
    C["v1"] = al.tile([128, NRT * VW], ADT, tag="v1")
    C["ot"] = al.tile([128, NPAIR * ROWS], PDT, tag="ot")
    C["wq01"] = al.tile([128, NKD * 128], XDT, tag="wq01")
    C["wq23"] = al.tile([128, NKD * 128], XDT, tag="wq23")
    C["wk01"] = al.tile([128, NKD * 128], XDT, tag="wk01")
    C["wk23"] = al.tile([128, NKD * 128], XDT, tag="wk23")
    C["wv"] = al.tile([128, NKD * 256], XDT, tag="wv")
    C["wp"] = al.tile([128, NPAIR * DM], PDT, tag="wp")
    C["bq01"] = al.tile([128, 1], F32, tag="bq01")
    C["bq23"] = al.tile([128, 1], F32, tag="bq23")
    C["bk01"] = al.tile([128, 1], F32, tag="bk01")
    C["bk23"] = al.tile([128, 1], F32, tag="bk23")
    C["bvrow"] = al.tile([1, 256], F32R, tag="bvrow")
    C["ones1"] = al.tile([1, 128], F32, tag="ones1")
    C["triu2"] = al.tile([128, 256], ADT, tag="triu2")

    for name, dst in (("wq01", wq01_d), ("wq23", wq23_d), ("wk01", wk01_d),
                      ("wk23", wk23_d), ("wv", wv_d), ("wp", wp_d),
                      ("bq01", bq01_d), ("bq23", bq23_d), ("bk01", bk01_d),
                      ("bk23", bk23_d), ("bvrow", bvrow_d), ("triu2", triu2_d)):
        nc.sync.dma_start(C[name][:], dst[:])
    nc.vector.memset(C["ones1"][:], 1.0)
    # ones columns of the V blocks (col 64 and 129 of each 130-block)
    for vp in ("v0", "v1"):
        vb = C[vp].rearrange("p (i w) -> p i w", w=VW)
        nc.vector.memset(vb[:, :, HD], 1.0)
        nc.vector.memset(vb[:, :, 2 * HD + 1], 1.0)
    # pre-loop xt load (iterations 2+ use the in-body prefetches instead)
    xts_v = C["xts"].rearrange("p (k r) -> p k r", k=NKD)
    xt_dv = xt_d.rearrange("p (k r) -> p k r", k=NKD)
    for rc in range(NQC):
        nc.gpsimd.dma_start(xts_v[:, :, rc * QCH:(rc + 1) * QCH],
                            xt_dv[:, :, rc * QCH:(rc + 1) * QCH])
    return C


def _emit_body(nc, al, C, pre=False, parts=("a", "attn", "proj")):
    xts = C["xts"].rearrange("p (k r) -> p k r", k=NKD)
    xt_dv = C["xt_d"].rearrange("p (k r) -> p k r", k=NKD)
    qtb = C["qtB"].rearrange("p (pair r) -> p pair r", pair=NPAIR)
    ktb = C["ktB"].rearrange("p (pair r) -> p pair r", pair=NPAIR)
    otv = C["ot"].rearrange("p (pair r) -> p pair r", pair=NPAIR)
    wpv = C["wp"].rearrange("p (pair o) -> p pair o", pair=NPAIR)
    wvv = C["wv"].rearrange("p (k c) -> p k c", k=NKD)
    triu2v = C["triu2"].rearrange("p (h q) -> p h q", h=2)
    vblk = [C["v0"].rearrange("p (i w) -> p i w", w=VW),
            C["v1"].rearrange("p (i w) -> p i w", w=VW)]

    def xt_load(rc):
        nc.gpsimd.dma_start(xts[:, :, rc * QCH:(rc + 1) * QCH],
                            xt_dv[:, :, rc * QCH:(rc + 1) * QCH])

    def wview(t):
        return t.rearrange("p (k c) -> p k c", k=NKD)

    def a_qk(rc, which, tag="ps"):
        w01, w23 = (C["wq01"], C["wq23"]) if which == "q" else (C["wk01"], C["wk23"])
        b01, b23 = (C["bq01"], C["bq23"]) if which == "q" else (C["bk01"], C["bk23"])
        dst = qtb if which == "q" else ktb
        ps = al.tile([128, 2 * QCH], F32, tag=tag)
        cs = rc * QCH
        if STAGE_A_FP8:
            for kp in range(NKD // 2):
                xr = xts[:, 2 * kp:2 * kp + 2, cs:cs + QCH]
                st, sp = kp == 0, kp == NKD // 2 - 1
                nc.tensor.matmul(ps[:, 0:QCH], wview(w01)[:, 2 * kp:2 * kp + 2, :],
                                 xr, start=st, stop=sp, perf_mode=DR)
                nc.tensor.matmul(ps[:, QCH:], wview(w23)[:, 2 * kp:2 * kp + 2, :],
                                 xr, start=st, stop=sp, perf_mode=DR)
        else:
            for k in range(NKD):
                xr = xts[:, k, cs:cs + QCH]
                st, sp = k == 0, k == NKD - 1
                nc.tensor.matmul(ps[:, 0:QCH], w01[:, k * 128:(k + 1) * 128],
                                 xr, start=st, stop=sp)
                nc.tensor.matmul(ps[:, QCH:], w23[:, k * 128:(k + 1) * 128],
                                 xr, start=st, stop=sp)
        nc.vector.tensor_scalar_add(dst[:, 0, cs:cs + QCH], ps[:, 0:QCH], b01[:])
        nc.vector.tensor_scalar_add(dst[:, 1, cs:cs + QCH], ps[:, QCH:], b23[:])

    def a_v(rc, h):
        ps = al.tile([128, 2 * QCH], F32, tag="ps")
        for j in range(2):
            rt = rc * 4 + h * 2 + j
            base = j * QCH
            if STAGE_A_FP8:
                for kp in range(NKD // 2):
                    xl = xts[:, 2 * kp:2 * kp + 2, rt * 128:(rt + 1) * 128]
                    nc.tensor.matmul(ps[:, base:base + 256],
                                     xl, wvv[:, 2 * kp:2 * kp + 2, :],
                                     start=kp == 0, stop=False, perf_mode=DR)
            else:
                for k in range(NKD):
                    nc.tensor.matmul(ps[:, base:base + 256],
                                     xts[:, k, rt * 128:(rt + 1) * 128],
                                     wvv[:, k, :], start=k == 0, stop=False)
            nc.tensor.matmul(ps[:, base:base + 256],
                             C["ones1"][:, 0:128].bitcast(F32R),
                             C["bvrow"][:], start=False, stop=True)
        # drain both row tiles for each pair: src [p, 2rt, 2head, 64]
        pv = ps.rearrange("p (j x) -> p j x", j=2)
        for pair in range(2):
            src = pv[:, :, pair * 128:(pair + 1) * 128].rearrange(
                "p j (h c) -> p j h c", h=2)
            dstb = vblk[pair][:, rc * 4 + h * 2: rc * 4 + h * 2 + 2, :].rearrange(
                "p j (h w) -> p j h w", h=2)[:, :, :, 0:HD]
            nc.vector.tensor_copy(dstb, src)

    def proj(qc, j):
        q0 = qc * QCH + j * 128
        psp = al.tile([128, 2 * QCH], F32, tag="ps")
        if PROJ_FP8:
            for ct in range(2):
                nc.tensor.matmul(psp[:, ct * QCH:(ct + 1) * QCH],
                                 otv[:, :, q0:q0 + 128],
                                 wpv[:, :, ct * QCH:(ct + 1) * QCH], perf_mode=DR)
        else:
            # pair-outer so the ot stationary is reused across both ct MMs
            for pair in range(2):
                for ct in range(2):
                    nc.tensor.matmul(psp[:, ct * QCH:(ct + 1) * QCH],
                                     otv[:, pair, q0:q0 + 128],
                                     wpv[:, pair, ct * QCH:(ct + 1) * QCH],
                                     start=pair == 0, stop=pair == 1)
        osb = al.tile([128, DM], BF16, tag="osb")
        if qc >= 2:
            nc.scalar.copy(osb[:], psp[:])
        else:
            nc.vector.tensor_copy(osb[:], psp[:])
        nc.sync.dma_start(C["out_d"][q0:q0 + 128, :], osb[:])

    def attn_scores(s, qc, kt):
        r = kt * KTILE - qc * QCH
        s0 = max(0, r)
        kcol = kt * KTILE
        qlo = qc * QCH
        ps = al.tile([128, 2 * QCH], F32, tag="ps")
        from contextlib import nullcontext
        with (al.tc.high_priority(offset=SCORE_PRI) if SCORE_PRI else nullcontext()):
            nc.tensor.matmul(ps[:, s0:QCH], ktb[0:HD, s, kcol:kcol + KTILE],
                             qtb[0:HD, s, qlo + s0:qlo + QCH])
            nc.tensor.matmul(ps[:, QCH + s0:], ktb[HD:128, s, kcol:kcol + KTILE],
                             qtb[HD:128, s, qlo + s0:qlo + QCH])
        ea = al.tile([128, 2 * QCH], ADT, tag="ea")
        if NO_EXP:
            nc.vector.memset(ea[:, 0:1], 1.0)  # diagnostic: tiny placeholder write
        else:
            pv = ps.rearrange("p (h q) -> p h q", h=2)[:, :, s0:]
            ev = ea.rearrange("p (h q) -> p h q", h=2)[:, :, s0:]
            nc.scalar.activation(ev, pv, EXP, scale=0.125)
            if r >= 0:  # diagonal tile: triangular mask on the 128-col bands
                band = ea.rearrange("p (h q) -> p h q", h=2)[:, :, s0:s0 + KTILE]
                nc.vector.tensor_mul(band, band, triu2v)
        return ea, s0

    def attn_av(s, qc, kt, ea, s0, pso):
        if NO_AV:
            return
        st = kt == 0
        sp = kt == (qc + 1) * 4 - 1
        nc.tensor.matmul(pso[:, s0:QCH], vblk[s][:, kt, 0:HD + 1],
                         ea[:, s0:QCH], start=st, stop=sp)
        nc.tensor.matmul(pso[:, QCH + s0:], vblk[s][:, kt, HD + 1:VW],
                         ea[:, QCH + s0:], start=st, stop=sp)

    def fin_recip(s, pso):
        rc2 = al.tile([1, 2 * QCH], F32, tag="rc2")
        nc.vector.reciprocal(rc2[:], pso[HD:HD + 1, :])
        return rc2

    def fin_bcast(s, rc2):
        bc = al.tile([HD, 2 * QCH], F32, tag="bc")
        nc.gpsimd.partition_broadcast(bc[:], rc2[:])
        return bc

    def fin_muls(s, qc, pso, bc):
        qlo = qc * QCH
        nc.vector.tensor_mul(otv[0:HD, s, qlo:qlo + QCH], pso[0:HD, 0:QCH],
                             bc[:, 0:QCH])
        nc.vector.tensor_mul(otv[HD:128, s, qlo:qlo + QCH], pso[0:HD, QCH:],
                             bc[:, QCH:])

    if pre:
        # pre-loop: stage A rc0 for the first iteration (later iterations
        # get rc0 from the previous body's round 3 + tail)
        if "a" in parts:
            a_qk(0, "q")
            a_qk(0, "k")
            a_v(0, 0)
            a_v(0, 1)
        return

    # ---- rounds (stage A rc0 was emitted pre-loop / by the previous
    # iteration's round 3 + tail) ----
    for r in range(NQC):
        nkt = (r + 1) * 4
        aux = []
        if "a" in parts:
            if r == 2:  # prefetch next iteration's xt (rc0/rc1 readers done)
                aux.append(lambda: xt_load(0))
                aux.append(lambda: xt_load(1))
            if r == 3:
                aux.append(lambda: xt_load(2))
                aux.append(lambda: xt_load(3))
            if r + 1 < NQC:
                rc = r + 1
                aux.append(lambda rc=rc: a_qk(rc, "q"))
                aux.append(lambda rc=rc: a_qk(rc, "k"))
                aux.append(lambda rc=rc: a_v(rc, 0))
                aux.append(lambda rc=rc: a_v(rc, 1))
        if "proj" in parts and r >= 1:
            for j in range(4):
                aux.append(lambda qc=r - 1, j=j: proj(qc, j))
        if "a" in parts and r == 3:
            # next iteration's qT/kT rc0; lands late (i>=~10) so this
            # iteration's kt0..3 reads of qtB/ktB are already in the PE past
            aux.append(lambda: a_qk(0, "q"))
            aux.append(lambda: a_qk(0, "k"))
        if "attn" not in parts:
            for fn in aux:
                fn()
            continue
        # sequential streams: stream 0's whole chunk, then stream 1's.
        # The shared 2-buf ring then gives a depth-2 score pipeline within
        # each chunk, and each stream's finalize chain (recip -> broadcast ->
        # muls) hides under the other stream's units.
        naux = len(aux)
        emitted = 0
        total_units = 2 * nkt
        unit = 0
        for s in range(2):
            pso = al.tile([HD + 1, 2 * QCH], F32, tag="pso")
            pend = []  # scores run 2 units ahead of their AV matmuls
            for i in range(nkt):
                pend.append((i, attn_scores(s, r, i)))
                if len(pend) > 2:
                    j, (ea, s0) = pend.pop(0)
                    attn_av(s, r, j, ea, s0, pso)
                unit += 1
                while emitted < naux * unit // total_units:
                    aux[emitted]()
                    emitted += 1
            for j, (ea, s0) in pend:
                attn_av(s, r, j, ea, s0, pso)
            if not NO_AV:
                rc2 = fin_recip(s, pso)
                bc = fin_bcast(s, rc2)
                fin_muls(s, r, pso, bc)

    # ---- tail: next iteration's V rc0, proj of last chunk ----
    if "a" in parts:
        a_v(0, 0)
        a_v(0, 1)
    if "proj" in parts:
        for j in range(4):
            proj(NQC - 1, j)


def _maybe_enable_ldw_opt():
    """Flip walrus's --enable-ldw-opt to true (background weight buffer /
    Ldweights pull-ahead) for this process's compiles."""
    if os.environ.get("K_LDW_OPT", "0") != "1":
        return
    from concourse import bass_utils as _bu
    if getattr(_bu, "_k_ldw_patched", False):
        return
    _orig = _bu.run_command

    def _rc(cmd, *a, **k):
        if isinstance(cmd, list):
            cmd = ["--enable-ldw-opt=true" if c == "--enable-ldw-opt=false" else c
                   for c in cmd]
        return _orig(cmd, *a, **k)

    _bu.run_command = _rc
    _bu._k_ldw_patched = True


def build_module(repeat=1, loop_n=0, parts=("a", "attn", "proj"), pre_parts=()):
    _maybe_enable_ldw_opt()
    nc = bacc.Bacc("TRN2", target_bir_lowering=False, debug=False,
                   enable_asserts=True, num_devices=NCORES)

    def din(name, shape, dt):
        return nc.dram_tensor(name, shape, dt, kind="ExternalInput").ap()

    aps = (
        din("xt", [128, NKD * ROWS], XDT),
        din("wq01", [128, NKD * 128], XDT),
        din("wq23", [128, NKD * 128], XDT),
        din("wk01", [128, NKD * 128], XDT),
        din("wk23", [128, NKD * 128], XDT),
        din("wv", [128, NKD * 256], XDT),
        din("wp", [128, NPAIR * DM], PDT),
        din("bq01", [128, 1], F32),
        din("bq23", [128, 1], F32),
        din("bk01", [128, 1], F32),
        din("bk23", [128, 1], F32),
        din("bvrow", [1, 256], F32R),
        din("triu2", [128, 256], ADT),
        nc.dram_tensor("out", [ROWS, DM], BF16, kind="ExternalOutput").ap(),
    )
    with tile.TileContext(nc) as tc:
        with tc.tile_pool(name="pers", bufs=1) as pers, \
             tc.tile_pool(name="work", bufs=4) as work, \
             tc.tile_pool(name="ps", bufs=2, space="PSUM") as psp:
            al = _Alloc(pers, work, psp)
            al.tc = tc
            consts = _emit_consts(nc, al, aps)
            _emit_body(nc, al, consts, pre=True, parts=parts)
            if loop_n:
                with tc.For_i(0, loop_n, 1):
                    _emit_body(nc, al, consts, parts=parts)
            else:
                for _ in range(repeat):
                    _emit_body(nc, al, consts, parts=parts)
    nc.compile()
    return nc


def _np_dt(dt):
    import ml_dtypes
    return {BF16: ml_dtypes.bfloat16, FP8: ml_dtypes.float8_e4m3,
            F32: np.float32, F32R: np.float32}[dt]


def _ktile_major(w, ncols):
    """[DM, ncols] -> [128, NKD*ncols] with w[kt*128+p, c] at [p, kt*ncols+c]."""
    return np.ascontiguousarray(
        w.reshape(NKD, 128, ncols).transpose(1, 0, 2).reshape(128, NKD * ncols))


def _host_prep(x, W_qkv, b_qkv, W_proj):
    x = np.asarray(x, np.float32)
    W_qkv = np.asarray(W_qkv, np.float32)
    b_qkv = np.asarray(b_qkv, np.float32)
    W_proj = np.asarray(W_proj, np.float32)
    xdt = _np_dt(XDT)
    adt = _np_dt(ADT)
    pdt = _np_dt(PDT)
    triu = np.triu(np.ones((128, 128), np.float32))
    triu2 = np.concatenate([triu, triu], axis=1).astype(adt)
    in_maps = []
    for c in range(NCORES):
        b = c // 4
        h0 = (c % 4) * 4
        q0 = h0 * HD          # first q column of the 4 heads
        xt = _ktile_major(np.ascontiguousarray(x[b].T), ROWS)
        in_maps.append({
            "xt": xt.astype(xdt),
            "wq01": _ktile_major(W_qkv[:, q0:q0 + 128], 128).astype(xdt),
            "wq23": _ktile_major(W_qkv[:, q0 + 128:q0 + 256], 128).astype(xdt),
            "wk01": _ktile_major(W_qkv[:, DM + q0:DM + q0 + 128], 128).astype(xdt),
            "wk23": _ktile_major(W_qkv[:, DM + q0 + 128:DM + q0 + 256], 128).astype(xdt),
            "wv": _ktile_major(W_qkv[:, 2 * DM + q0:2 * DM + q0 + 256], 256).astype(xdt),
            "wp": np.ascontiguousarray(
                W_proj[q0:q0 + 256, :].reshape(2, 128, DM).transpose(1, 0, 2)
                .reshape(128, 2 * DM)).astype(pdt),
            "bq01": np.ascontiguousarray(b_qkv[q0:q0 + 128, None]),
            "bq23": np.ascontiguousarray(b_qkv[q0 + 128:q0 + 256, None]),
            "bk01": np.ascontiguousarray(b_qkv[DM + q0:DM + q0 + 128, None]),
            "bk23": np.ascontiguousarray(b_qkv[DM + q0 + 128:DM + q0 + 256, None]),
            "bvrow": np.ascontiguousarray(b_qkv[None, 2 * DM + q0:2 * DM + q0 + 256]),
            "triu2": triu2,
        })
    return in_maps


class _Runner:
    """Compile once, execute many times (mirrors bass2jax.run_bass_via_pjrt)."""

    def __init__(self, nc):
        import jax
        from jax.sharding import Mesh, PartitionSpec
        from jax.experimental.shard_map import shard_map
        from concourse import bass2jax
        from concourse import mybir as _mybir

        bass2jax.install_neuronx_cc_hook()
        self.jax = jax
        in_names, out_names, out_avals, zero_shapes = [], [], [], []
        partition_name = nc.partition_id_tensor.name if nc.partition_id_tensor else None
        for alloc in nc.m.functions[0].allocations:
            if not isinstance(alloc, _mybir.MemoryLocationSet):
                continue
            name = alloc.memorylocations[0].name
            if alloc.kind == "ExternalInput":
                if name != partition_name:
                    in_names.append(name)
            elif alloc.kind == "ExternalOutput":
                shape = tuple(alloc.tensor_shape)
                dtype = _mybir.dt.np(alloc.dtype)
                out_names.append(name)
                out_avals.append(jax.core.ShapedArray(shape, dtype))
                zero_shapes.append((shape, dtype))
        self.in_names = in_names
        self.out_names = out_names
        self.out_avals = out_avals
        self.zero_shapes = zero_shapes
        n_params = len(in_names)
        n_outs = len(out_avals)
        all_in_names = in_names + out_names + ([partition_name] if partition_name else [])

        def _body(*args):
            operands = list(args)
            if partition_name is not None:
                operands.append(bass2jax.partition_id_tensor())
            outs = bass2jax._bass_exec_p.bind(
                *operands,
                out_avals=tuple(out_avals),
                in_names=tuple(all_in_names),
                out_names=tuple(out_names),
                lowering_input_output_aliases=(),
                sim_require_finite=True,
                sim_require_nnan=True,
                nc=nc,
            )
            return tuple(outs)

        devices = jax.devices()[:NCORES]
        mesh = Mesh(np.asarray(devices), ("core",))
        self.mesh = mesh
        self.pspec = PartitionSpec("core")
        in_specs = (PartitionSpec("core"),) * (n_params + n_outs)
        out_specs = (PartitionSpec("core"),) * n_outs
        self.donate = tuple(range(n_params, n_params + n_outs))
        self.sharded = jax.jit(
            shard_map(_body, mesh=mesh, in_specs=in_specs, out_specs=out_specs,
                      check_rep=False),
            donate_argnums=self.donate, keep_unused=True)

    def concat_inputs(self, in_maps):
        return [np.concatenate([np.asarray(m[name]) for m in in_maps], axis=0)
                for name in self.in_names]

    def zeros(self):
        return [np.zeros((NCORES * s[0], *s[1:]), d) for (s, d) in self.zero_shapes]

    def run(self, concat_in):
        outs = self.sharded(*concat_in, *self.zeros())
        outs = self.jax.block_until_ready(outs)
        return outs

    def device_inputs(self, concat_in):
        from jax.sharding import NamedSharding
        sh = NamedSharding(self.mesh, self.pspec)
        return [self.jax.device_put(a, sh) for a in concat_in]

    def device_zeros(self):
        import jax.numpy as jnp
        from jax.sharding import NamedSharding
        sh = NamedSharding(self.mesh, self.pspec)
        return [jnp.zeros((NCORES * s[0], *s[1:]), d, device=sh)
                for (s, d) in self.zero_shapes]

    def run_device(self, dev_in):
        outs = self.sharded(*dev_in, *self.device_zeros())
        outs = self.jax.block_until_ready(outs)
        return outs

    def split_out(self, outs):
        res = {}
        for i, name in enumerate(self.out_names):
            res[name] = np.asarray(outs[i]).reshape(NCORES, *self.out_avals[i].shape)
        return res


_CACHE = {}


def _get_runner(repeat=1, loop_n=0, parts=("a", "attn", "proj"), pre_parts=()):
    parts = tuple(parts) if parts else ("a", "attn", "proj")
    key = ("runner", repeat, loop_n, STAGE_A_FP8, PROJ_FP8, parts)
    if key not in _CACHE:
        nc = build_module(repeat=repeat, loop_n=loop_n, parts=parts)
        _CACHE[key] = _Runner(nc)
    return _CACHE[key]


def kernel(x, W_qkv, b_qkv, W_proj, b_proj):
    runner = _get_runner(repeat=1)
    in_maps = _host_prep(x, W_qkv, b_qkv, W_proj)
    concat_in = runner.concat_inputs(in_maps)
    outs = runner.run(concat_in)
    parts = runner.split_out(outs)["out"]  # [8, 2048, 1024] bf16
    parts = parts.astype(np.float32)
    b_proj = np.asarray(b_proj, np.float32)
    full = np.stack([parts[4 * b:4 * b + 4].sum(axis=0) + b_proj[None, :]
                     for b in range(B)])
    return full.astype(np.float32)
